# revision 1
# baseline (speedup 1.0000x reference)
"""Trainium2 Bass kernel for a quantized BasicBlock (QConv3x3 -> RangeNorm ->
QConv3x3 -> RangeNorm -> quantized residual add).

Sharding: data-parallel over batch (8 images per core across 8 cores);
weights replicated; per-tensor quantization min/max and per-channel
range-norm stats are combined across cores with small AllReduce collectives.

Per core:
  - x streamed once (8 MiB, 1-image chunks on 2 DMA queues); per-chunk max
    on Pool, min via a pairwise fp16 tensor_tensor tree on DVE (scratch in
    the not-yet-written qxpad interiors), fp32->fp16 convert on ACT into a
    resident SBUF copy. Global min/max via one AllReduce(max) of
    (-min, max); then k = rne((x-xmin)/s) on ACT (fp32->int32 RNE) from the
    fp16 copy, qx = s*k + xmin stored fp16 in a zero-padded [34,34] layout.
  - w1/w2 staged in borrowed idle buffers (out1 / x16) in 4 bus pieces;
    quantized the same way; w2 prepped inside conv1's shadow.
  - conv3x3 = 18 accumulating PE matmuls (2 ci blocks x 3x3 taps) per
    [co_block, half-image] PSUM tile; fp16 operands, fp32 PSUM. Image-edge
    taps skip the rows/cols that only multiply halo zeros (~7% fewer PE
    cycles). Per-half-tile min/max/sum stats feed RangeNorm.
  - RangeNorm + quantize fused into per-channel affine + int32 round; the
    quantizer's min/max derives analytically from per-channel conv min/max;
    channel stats combined with AllReduce(add) + AllReduce(max) of
    (sums, -min, max). z-pads built in halves so conv2 starts on the top
    half (subtile deps).
  - conv2 output reuses the conv1 output buffer (dead after z is built).
  - final: out = qx + dequant(round(A2*conv2 + B2)); round on ACT, dequant
    in place in k (DVE for the latency-critical end chunks, Pool in the
    middle), residual add on DVE into slots carved from the dead x16
    buffer, 2-queue chunked DMAs out.
"""

import os
import numpy as np

N_CORES = 8
NLOC = 8            # images per core
C = 256
P = 128
HW = 1024           # 32*32
PAD = 34            # 32+2
EPS = 1e-8
QMAX = 255.0
N_TOTAL = 64 * 32 * 32          # range-norm n (global batch)
C_N = float(1.0 / np.sqrt(2.0 * np.log(N_TOTAL)))

_cached_nc = None


def _build(sim_single=False, no_collectives=False):
    """sim_single=True builds a 1-core variant with collectives replaced by
    a stand-in DMA — numerically wrong across cores but structurally
    identical, for TimelineSim cost-model analysis. no_collectives=True keeps
    8 cores but swaps collectives for local DMAs (timing A/B only)."""
    import concourse.bass as bass
    import concourse.mybir as mybir
    from concourse import bacc, tile
    import concourse.bass_isa as bass_isa

    dt = mybir.dt
    F32, F16, I32 = dt.float32, dt.float16, dt.int32
    AX = mybir.AxisListType.X
    AXY = mybir.AxisListType.XY
    OP = mybir.AluOpType
    ACTF = mybir.ActivationFunctionType.Identity
    RMAX = bass_isa.ReduceOp.max

    nc = bacc.Bacc("TRN2", target_bir_lowering=False, debug=False,
                   num_devices=(1 if sim_single else N_CORES))

    local_cc = sim_single or no_collectives

    def _flat(ap):
        names = "abcde"[:len(ap.shape)]
        if len(names) == 1:
            return ap
        spec = " ".join(names)
        return ap.rearrange(f"{spec} -> ({spec})")

    def allreduce(op, snd, rcv):
        """AllReduce over all cores, elementwise `op`. snd/rcv same shape."""
        if local_cc:
            nc.sync.dma_start(_flat(rcv)[None, :], _flat(snd)[None, :])
        else:
            nc.gpsimd.collective_compute(
                "AllReduce", op,
                replica_groups=[list(range(N_CORES))],
                ins=[snd.opt()], outs=[rcv.opt()])

    def allgather(snd, rcv, nelem):
        """AllGather snd (nelem elems) -> rcv [N_CORES, nelem]."""
        if local_cc:
            nc.sync.dma_start(_flat(rcv)[0:nelem][None, :], _flat(snd)[None, :])
        else:
            nc.gpsimd.collective_compute(
                "AllGather", mybir.AluOpType.bypass,
                replica_groups=[list(range(N_CORES))],
                ins=[snd.opt()], outs=[rcv.opt()])

    x_d = nc.dram_tensor("x", [NLOC, C, HW], F32, kind="ExternalInput")
    eye_d = nc.dram_tensor("eye", [P, P], dt.float16, kind="ExternalInput")
    w1_d = nc.dram_tensor("w1t", [C, C, 9], F32, kind="ExternalInput")
    w2_d = nc.dram_tensor("w2t", [C, C, 9], F32, kind="ExternalInput")
    g1_d = nc.dram_tensor("gamma1", [C], F32, kind="ExternalInput")
    b1_d = nc.dram_tensor("beta1", [C], F32, kind="ExternalInput")
    g2_d = nc.dram_tensor("gamma2", [C], F32, kind="ExternalInput")
    b2_d = nc.dram_tensor("beta2", [C], F32, kind="ExternalInput")
    out_d = nc.dram_tensor("out", [NLOC, C, HW], F32, kind="ExternalOutput")

    with tile.TileContext(nc) as tc:
        with tc.tile_pool(name="consts", bufs=1) as cp, \
             tc.tile_pool(name="dram", bufs=1, space="DRAM") as dp, \
             tc.tile_pool(name="psum", bufs=8, space="PSUM") as pp, \
             tc.tile_pool(name="xsp", bufs=8) as xsp, \
             tc.tile_pool(name="ktmp", bufs=3) as kp:

            # ---------- persistent tiles ----------
            qxpad = cp.tile([P, 2 * NLOC, PAD, PAD], F16, tag="qxpad")
            x16 = cp.tile([P, 2, NLOC, HW], F16, tag="x16", name="x16")
            out1 = [cp.tile([P, NLOC, HW], F16, tag=f"out1_{a}",
                            name=f"out1_{a}") for a in (0, 1)]
            wl1 = [cp.tile([P, 9, C], F16, tag=f"wl1_{a}", name=f"wl1_{a}")
                   for a in (0, 1)]
            wl2 = [cp.tile([P, 9, C], F16, tag=f"wl2_{a}", name=f"wl2_{a}")
                   for a in (0, 1)]
            zb = [cp.tile([P, PAD, PAD], F16, tag=f"zb_{i}", name=f"zb_{i}")
                  for i in range(2)]

            def scal(tag, cols=1):
                return cp.tile([P, cols], F32, tag=tag, name=tag)

            # ---------- t0: halo zeroing on DVE (cheap, idle at head) ----------
            # qxpad halo: rows 0/33 for all 16 sub-images, cols 0/33.
            nc.vector.memset(qxpad[:, :, 0, :], 0.0)
            nc.vector.memset(qxpad[:, :, 33, :], 0.0)
            nc.vector.memset(qxpad[:, :, 1:33, 0], 0.0)
            nc.vector.memset(qxpad[:, :, 1:33, 33], 0.0)
            # identity for the PE residual-add at the tail
            eye = cp.tile([P, P], F16, tag="eye", name="eye")
            nc.gpsimd.dma_start(eye[:, :], eye_d.ap()[:, :])

            # =====================================================
            # x pass 1: stream 1-image chunks on 2 queues; min on DVE,
            # max on Pool, fp32->fp16 convert on ACT into resident x16.
            # w1 DMA interleaved mid-stream on both queues.
            # =====================================================
            xg = cp.tile([1, 16], F32, tag="xg")       # chunk maxima (scalar)
            zero = scal("zero")
            one = scal("one")
            nc.vector.memset(zero[:], 0.0)
            nc.vector.memset(one[:], 1.0)

            # weight staging borrows idle persistent buffers: w1 lives in
            # out1 (first written at conv1), w2 in x16 (released by pass 2).
            # Each entry is (f32 view, i32 view) per ci block.
            _o1f = [out1[a].bitcast(F32).rearrange("p n s -> p (n s)")
                    for a in (0, 1)]
            _o1i = [out1[a].bitcast(I32).rearrange("p n s -> p (n s)")
                    for a in (0, 1)]
            _x16f = x16.bitcast(F32).rearrange("p a n s -> p (a n s)")
            _x16i = x16.bitcast(I32).rearrange("p a n s -> p (a n s)")
            NW = C * 9
            wstage = [
                [(_o1f[a][:, 0:NW], _o1i[a][:, 0:NW]) for a in (0, 1)],
                [(_x16f[:, a * NW:(a + 1) * NW], _x16i[:, a * NW:(a + 1) * NW])
                 for a in (0, 1)],
            ]
            WPC = 4          # weight DMA pieces per ci block (bus granularity)

            def w_load(idx, w_dram):
                for a in (0, 1):
                    t = wstage[idx - 1][a][0]
                    src = w_dram.ap()[a * P:(a + 1) * P, :, :].rearrange(
                        "p c q -> p (c q)")
                    step = NW // WPC
                    for piece in range(WPC):
                        lo = piece * step
                        eng = nc.sync if (piece + a) % 2 == 0 else nc.scalar
                        eng.dma_start(t[:, lo:lo + step], src[:, lo:lo + step])

            def w_stat_one(idx, i, wmn_p, wmx_p):
                """One of the four min/max reduces for the staged w blocks."""
                a, is_max = i // 2, i % 2
                t = wstage[idx - 1][a][0]
                if is_max:
                    nc.vector.tensor_reduce(wmx_p[:, a:a + 1], t, AX, OP.max)
                else:
                    nc.vector.tensor_reduce(wmn_p[:, a:a + 1], t, AX, OP.min,
                                            negate=True)

            def w_stats(idx):
                """Emit min/max reduces for the staged w blocks."""
                wmn_p = cp.tile([P, 2], F32, tag=f"wmnp{idx}", name=f"wmnp{idx}")
                wmx_p = cp.tile([P, 2], F32, tag=f"wmxp{idx}", name=f"wmxp{idx}")
                for i in range(4):
                    w_stat_one(idx, i, wmn_p, wmx_p)
                return wmn_p, wmx_p

            def w_scalars(idx, wmn_p, wmx_p):
                def s1(tag):
                    return cp.tile([P, 1], F32, tag=f"{tag}{idx}", name=f"{tag}{idx}")
                pn = s1("wpn")
                px = s1("wpx")
                nc.vector.tensor_reduce(pn[:], wmn_p[:, :], AX, OP.max)
                nc.vector.tensor_reduce(px[:], wmx_p[:, :], AX, OP.max)
                nmn = s1("wnmn")
                gmx = s1("wgmx")
                nc.gpsimd.partition_all_reduce(nmn[:], pn[:], P, RMAX)
                nc.gpsimd.partition_all_reduce(gmx[:], px[:], P, RMAX)
                gmn = s1("wgmn")
                nc.vector.tensor_scalar(gmn[:], nmn[:], -1.0, None, OP.mult)
                rng = s1("wrng")
                nc.vector.tensor_tensor(rng[:], gmx[:], nmn[:], OP.add)
                s = s1("ws_")
                nc.vector.tensor_scalar(s[:], rng[:], 1.0 / QMAX, EPS, OP.mult, OP.max)
                inv = s1("winv")
                nc.vector.reciprocal(inv[:], s[:])
                bias = s1("wbias")
                nc.vector.tensor_tensor(bias[:], nmn[:], inv[:], OP.mult)
                return s, inv, bias, gmn

            def w_rounds(idx, inv, bias):
                for a in (0, 1):
                    tf, ti = wstage[idx - 1][a]
                    nc.scalar.activation(ti, tf, ACTF,
                                         bias=bias[:, 0:1], scale=inv[:, 0:1])

            def w_relayout_one(idx, wl, s, gmn, a):
                kv = wstage[idx - 1][a][1]
                nc.vector.tensor_scalar(
                    wl[a][:, :, :],
                    kv.rearrange("p (c q) -> p q c", q=9),
                    s[:, 0:1], gmn[:, 0:1], OP.mult, OP.add)

            def w_relayout(idx, wl, s, gmn):
                for a in (0, 1):
                    w_relayout_one(idx, wl, s, gmn, a)

            w1mn = cp.tile([P, 2], F32, tag="wmnp1", name="wmnp1")
            w1mx = cp.tile([P, 2], F32, tag="wmxp1", name="wmxp1")

            # x-min via a pairwise tensor_tensor min tree (fp16 levels run at
            # 2x on DVE, much cheaper than 16 full TensorReduces); tree
            # scratch lives in qxpad interiors (unwritten until pass 2).
            def tv(i):
                return qxpad[:, i, 1:33, 1:33]

            xcs = []

            def x_chunk(j, a):
                c = 2 * j + a
                t = xsp.tile([P, HW], F32, tag="xs")
                eng = nc.sync if a == 0 else nc.scalar
                eng.dma_start(t[:, :], x_d.ap()[j, a * P:(a + 1) * P, :])
                nc.scalar.activation(x16[:, a, j, :], t[:, :], ACTF,
                                     bias=zero[:, 0:1], scale=one[:, 0:1])
                nc.gpsimd.tensor_reduce(xg[:, c:c + 1], t[:, :],
                                        mybir.AxisListType.XYZWC, OP.max)
                xcs.append(t)
                # leaf: pairwise min of consecutive arrivals (releases the
                # fp32 staging quickly, fp16 output for 2x upper levels)
                if len(xcs) % 2 == 0:
                    li = len(xcs) // 2 - 1
                    nc.vector.tensor_tensor(
                        tv(li),
                        xcs[-2].rearrange("p (y x) -> p y x", x=32),
                        xcs[-1].rearrange("p (y x) -> p y x", x=32), OP.min)

            # w1 first: small pieces on the bus, stats/scalars in early idle
            w_load(1, w1_d)
            for i in range(4):
                w_stat_one(1, i, w1mn, w1mx)
            w1s, w1inv, w1bias, w1gmn = w_scalars(1, w1mn, w1mx)

            for j in range(NLOC):
                for a in (0, 1):
                    x_chunk(j, a)

            # upper tree levels (all-fp16 TT = 2x); late leaves get shallow
            # paths so the final min lands right after chunk 15
            nc.vector.tensor_tensor(tv(8), tv(0), tv(1), OP.min)
            nc.vector.tensor_tensor(tv(9), tv(2), tv(3), OP.min)
            nc.vector.tensor_tensor(tv(10), tv(4), tv(5), OP.min)
            nc.vector.tensor_tensor(tv(11), tv(8), tv(9), OP.min)
            nc.vector.tensor_tensor(tv(12), tv(11), tv(10), OP.min)
            nc.vector.tensor_tensor(tv(13), tv(12), tv(6), OP.min)
            nc.vector.tensor_tensor(tv(14), tv(13), tv(7), OP.min)

            # ACT rounds queue behind the 16 converts; wl1 ready mid-stream
            w_rounds(1, w1inv, w1bias)

            # ---------- x stats finalize + AllReduce(max) ----------
            pmn = scal("xpmn")
            nc.vector.tensor_reduce(pmn[:], tv(14), AXY, OP.min, negate=True)
            xpack = scal("xpack", 2)
            nc.gpsimd.partition_all_reduce(xpack[:, 0:1], pmn[:], P, RMAX)
            lmx = cp.tile([1, 1], F32, tag="xlmx", name="xlmx")
            nc.vector.tensor_reduce(lmx[:], xg[:, :], AX, OP.max)
            nc.vector.tensor_scalar(xpack[0:1, 1:2], lmx[0:1, 0:1], 1.0, None,
                                    OP.mult)

            snd_x = dp.tile([2], F32, tag="snd_x")
            rcv_x = dp.tile([2], F32, tag="rcv_x",
                            addr_space=("Local" if local_cc else "Shared"))
            nc.sync.dma_start(snd_x[None, :], xpack[0:1, 0:2])
            allreduce(OP.max, snd_x, rcv_x)
            gx = cp.tile([P, 2], F32, tag="gx")
            nc.sync.dma_start(
                gx[:, :], rcv_x[None, :].broadcast_to([P, 2]))

            # w1 relayout on DVE fills the collective round-trip latency
            w_relayout(1, wl1, w1s, w1gmn)
            # zb halos on Pool (idle after its maxes; needed only at conv2)
            for z in zb:
                nc.gpsimd.memset(z[:, 0, :], 0.0)
                nc.gpsimd.memset(z[:, 33, :], 0.0)
                nc.gpsimd.memset(z[:, 1:33, 0], 0.0)
                nc.gpsimd.memset(z[:, 1:33, 33], 0.0)

            nxmin = scal("nxmin")
            xmax = scal("xmax")
            nc.vector.tensor_scalar(nxmin[:], gx[:, 0:1], 1.0, None, OP.mult)
            nc.vector.tensor_scalar(xmax[:], gx[:, 1:2], 1.0, None, OP.mult)
            xmin = scal("xmin")
            nc.vector.tensor_scalar(xmin[:], nxmin[:], -1.0, None, OP.mult)
            rngx = scal("rngx")
            nc.vector.tensor_tensor(rngx[:], xmax[:], nxmin[:], OP.add)
            sx = scal("sx")
            nc.vector.tensor_scalar(sx[:], rngx[:], 1.0 / QMAX, EPS, OP.mult, OP.max)
            invsx = scal("invsx")
            nc.vector.reciprocal(invsx[:], sx[:])
            biasx = scal("biasx")
            nc.vector.tensor_tensor(biasx[:], nxmin[:], invsx[:], OP.mult)

            # =====================================================
            # x pass 2 (from SBUF): k = rne((x-xmin)/s) on ACT, dequant
            # into padded fp16 qxpad on DVE. First chunks are 1 image to
            # shorten the pipeline fill before conv1's first matmul.
            # =====================================================
            CHUNKS = [(0, 1), (1, 1), (2, 2), (4, 2), (6, 2)]
            for n0, cnt in CHUNKS:
                for a in (0, 1):
                    k = kp.tile([P, cnt, HW], I32, tag=f"k{cnt}",
                                bufs=(4 if cnt == 1 else 3))
                    if n0 == 0:
                        # image 0 in halves: conv1's first tile starts after
                        # the top 17 interior rows (subtile deps)
                        for lo, hi, r0, r1 in ((0, 544, 1, 18),
                                               (544, 1024, 18, 33)):
                            nc.scalar.activation(
                                k[:, 0, lo:hi], x16[:, a, 0, lo:hi], ACTF,
                                bias=biasx[:, 0:1], scale=invsx[:, 0:1])
                            nc.vector.tensor_scalar(
                                qxpad[:, a * NLOC, r0:r1, 1:33],
                                k[:, 0, lo:hi].rearrange(
                                    "p (y x) -> p y x", x=32),
                                sx[:, 0:1], xmin[:, 0:1], OP.mult, OP.add)
                        continue
                    nc.scalar.activation(k[:, :, :], x16[:, a, n0:n0 + cnt, :],
                                         ACTF, bias=biasx[:, 0:1],
                                         scale=invsx[:, 0:1])
                    nc.vector.tensor_scalar(
                        qxpad[:, a * NLOC + n0:a * NLOC + n0 + cnt, 1:33, 1:33],
                        k.rearrange("p n (y x) -> p n y x", x=32),
                        sx[:, 0:1], xmin[:, 0:1], OP.mult, OP.add)

            # =====================================================
            # conv helper: 18 matmuls per [co_block, half] PSUM tile
            # =====================================================
            def conv(ns, in_pad_at, wl, outt, sums, mnt, mxt):
                for n in ns:
                    for cb in (0, 1):
                        for half in (0, 1):
                            ps = pp.tile([P, 512], F32, tag="ps")
                            i = 0
                            for a in (0, 1):
                                src = in_pad_at(n, a)
                                # ky=1 first: full 512 rows, valid start tap.
                                # The image-edge taps (ky=0 in the top half,
                                # ky=2 in the bottom) only multiply halo
                                # zeros in their first/last row - shave them.
                                for ky in (1, 0, 2):
                                    r0, o0 = half * 16 + ky, 0
                                    rows = 16
                                    if ky == 0 and half == 0:
                                        r0, o0, rows = 1, 32, 15
                                    elif ky == 2 and half == 1:
                                        rows = 15
                                    for kx in (1, 0, 2):
                                        # kx edge taps likewise only multiply
                                        # halo zeros in one column per row
                                        c0, x0, cols = kx, 0, 32
                                        if kx == 0:
                                            c0, x0, cols = 1, 1, 31
                                        elif kx == 2:
                                            cols = 31
                                        rhs = src[:, r0:r0 + rows,
                                                  c0:c0 + cols]
                                        pv = ps.rearrange(
                                            "p (y x) -> p y x", x=32)
                                        out = pv[:, o0 // 32:o0 // 32 + rows,
                                                 x0:x0 + cols]
                                        nc.tensor.matmul(
                                            out,
                                            wl[a][:, ky * 3 + kx,
                                                  cb * P:(cb + 1) * P],
                                            rhs, start=(i == 0), stop=(i == 17))
                                        i += 1
                            nc.scalar.activation(
                                outt[cb][:, n, half * 512:(half + 1) * 512],
                                ps[:], ACTF,
                                accum_out=sums[:, cb, n * 2 + half:n * 2 + half + 1])
                            h = n * 2 + half
                            nc.vector.tensor_reduce(
                                mnt[:, cb, h:h + 1],
                                outt[cb][:, n, half * 512:(half + 1) * 512],
                                AX, OP.min)
                            nc.vector.tensor_reduce(
                                mxt[:, cb, h:h + 1],
                                outt[cb][:, n, half * 512:(half + 1) * 512],
                                AX, OP.max)

            # =====================================================
            # range-norm stats AllReduce -> fused affine params
            # =====================================================
            def bn_params(idx, sums, mnt, mxt, gt, bt):
                # pk: cols 0:2 sums, 2:4 negated mins, 4:6 maxes — so the
                # min/max groups combine under a single AllReduce(max)
                pk = cp.tile([P, 6], F32, tag=f"pk{idx}", name=f"pk{idx}")
                for cb in (0, 1):
                    nc.vector.tensor_reduce(pk[:, cb:cb + 1], sums[:, cb, :], AX, OP.add)
                    nc.vector.tensor_reduce(pk[:, 2 + cb:3 + cb], mnt[:, cb, :], AX,
                                            OP.min, negate=True)
                    nc.vector.tensor_reduce(pk[:, 4 + cb:5 + cb], mxt[:, cb, :], AX, OP.max)
                snd_s = dp.tile([P, 2], F32, tag=f"snds{idx}", name=f"snds{idx}")
                snd_m = dp.tile([P, 4], F32, tag=f"sndm{idx}", name=f"sndm{idx}")
                rcv_s = dp.tile([P, 2], F32, tag=f"rcvs{idx}", name=f"rcvs{idx}",
                                addr_space=("Local" if local_cc else "Shared"))
                rcv_m = dp.tile([P, 4], F32, tag=f"rcvm{idx}", name=f"rcvm{idx}",
                                addr_space=("Local" if local_cc else "Shared"))
                nc.sync.dma_start(snd_s[:, :], pk[:, 0:2])
                nc.scalar.dma_start(snd_m[:, :], pk[:, 2:6])
                allreduce(OP.add, snd_s, rcv_s)
                allreduce(OP.max, snd_m, rcv_m)
                ssum = cp.tile([P, 2], F32, tag=f"ssum{idx}", name=f"ssum{idx}")
                stm = cp.tile([P, 4], F32, tag=f"stm{idx}", name=f"stm{idx}")
                nc.scalar.dma_start(ssum[:, :], rcv_s[:, :])
                nc.sync.dma_start(stm[:, :], rcv_m[:, :])
                smin = cp.tile([P, 2], F32, tag=f"smin{idx}", name=f"smin{idx}")
                nc.vector.tensor_scalar(smin[:], stm[:, 0:2], -1.0, None, OP.mult)
                smax = stm[:, 2:4]
                ssum = ssum[:, :]
                smin = smin[:, :]

                def t2(tag):
                    return cp.tile([P, 2], F32, tag=f"{tag}{idx}", name=f"{tag}{idx}")

                mean = t2("mean")
                nc.vector.tensor_scalar(mean[:], ssum, 1.0 / N_TOTAL, None, OP.mult)
                rng = t2("rng")
                nc.vector.tensor_tensor(rng[:], smax, smin, OP.subtract)
                sc = t2("sc")
                nc.vector.tensor_scalar(sc[:], rng[:], C_N, EPS, OP.mult, OP.add)
                inv = t2("inv")
                nc.vector.reciprocal(inv[:], sc[:])
                a_ = t2("a_")
                nc.vector.tensor_tensor(a_[:], gt[:], inv[:], OP.mult)
                am = t2("am")
                nc.vector.tensor_tensor(am[:], a_[:], mean[:], OP.mult)
                b_ = t2("b_")
                nc.vector.tensor_tensor(b_[:], bt[:], am[:], OP.subtract)
                lo = t2("lo")
                hi = t2("hi")
                nc.vector.tensor_tensor(lo[:], a_[:], smin, OP.mult)
                nc.vector.tensor_tensor(lo[:], lo[:], b_[:], OP.add)
                nc.vector.tensor_tensor(hi[:], a_[:], smax, OP.mult)
                nc.vector.tensor_tensor(hi[:], hi[:], b_[:], OP.add)
                lo2 = t2("lo2")
                hi2 = t2("hi2")
                nc.vector.tensor_tensor(lo2[:], lo[:], hi[:], OP.min)
                nc.vector.tensor_tensor(hi2[:], lo[:], hi[:], OP.max)
                def y1(tag):
                    return cp.tile([P, 1], F32, tag=f"{tag}{idx}", name=f"{tag}{idx}")
                pnl = y1("pnl")
                phi = y1("phi")
                nc.vector.tensor_reduce(pnl[:], lo2[:], AX, OP.min, negate=True)
                nc.vector.tensor_reduce(phi[:], hi2[:], AX, OP.max)
                nlom = y1("nlom")
                him = y1("him")
                nc.gpsimd.partition_all_reduce(nlom[:], pnl[:], P, RMAX)
                nc.gpsimd.partition_all_reduce(him[:], phi[:], P, RMAX)
                ymin = y1("ymin")
                nc.vector.tensor_scalar(ymin[:], nlom[:], -1.0, None, OP.mult)
                rngy = y1("rngy")
                nc.vector.tensor_tensor(rngy[:], him[:], nlom[:], OP.add)
                sy = y1("sy")
                nc.vector.tensor_scalar(sy[:], rngy[:], 1.0 / QMAX, EPS,
                                        OP.mult, OP.max)
                invsy = y1("invsy")
                nc.vector.reciprocal(invsy[:], sy[:])
                A = t2("A")
                nc.vector.tensor_scalar(A[:], a_[:], invsy[:, 0:1], None, OP.mult)
                B = t2("B")
                nc.vector.tensor_scalar(B[:], b_[:], ymin[:, 0:1], None, OP.subtract)
                nc.vector.tensor_scalar(B[:], B[:], invsy[:, 0:1], None, OP.mult)
                return A, B, sy[:, 0:1], ymin[:, 0:1]

            # ---------- conv1 (w2 prep + gamma/beta loads emitted after the
            # first two images so they run in conv1's engine-idle time) ----
            sums1 = cp.tile([P, 2, 16], F32, tag="sums1")
            mn1 = cp.tile([P, 2, 16], F16, tag="mn1")
            mx1 = cp.tile([P, 2, 16], F16, tag="mx1")
            nc.vector.memset(sums1[:, :, :], 0.0)
            qx_at = lambda n, a: qxpad[:, a * NLOC + n, :, :]
            conv(range(0, 2), qx_at, wl1, out1, sums1, mn1, mx1)

            w_load(2, w2_d)
            w2mn, w2mx = w_stats(2)
            w2s, w2inv, w2bias, w2gmn = w_scalars(2, w2mn, w2mx)
            w_rounds(2, w2inv, w2bias)
            w_relayout(2, wl2, w2s, w2gmn)
            # gamma/beta as [128, 2] (col = channel block); needed at bn1
            gb = {}
            for i, (nm, d) in enumerate(
                    (("g1", g1_d), ("b1", b1_d), ("g2", g2_d), ("b2", b2_d))):
                t = cp.tile([P, 2], F32, tag=f"gb_{nm}", name=f"gb_{nm}")
                eng = nc.sync if i % 2 == 0 else nc.scalar
                eng.dma_start(t[:], d.ap().rearrange("(a p) -> p a", p=P))
                gb[nm] = t

            conv(range(2, NLOC), qx_at, wl1, out1, sums1, mn1, mx1)
            A1, B1, sy1, ymin1 = bn_params(1, sums1, mn1, mx1, gb["g1"], gb["b1"])

            # ---------- z = quant(rangenorm(out1)); conv2 into out1 ----------
            sums2 = cp.tile([P, 2, 16], F32, tag="sums2")
            mn2 = cp.tile([P, 2, 16], F16, tag="mn2")
            mx2 = cp.tile([P, 2, 16], F16, tag="mx2")
            nc.vector.memset(sums2[:, :, :], 0.0)
            zpads = {}

            def get_z(n, a):
                if (n, a) not in zpads:
                    z = zb[(2 * n + a) % 2]
                    # built in halves: the top-half conv tiles only wait on
                    # the top 17 interior rows (subtile deps)
                    k = kp.tile([P, 1, HW], I32, tag="k1", bufs=4)
                    nc.scalar.activation(k[:, 0, 0:544], out1[a][:, n, 0:544],
                                         ACTF, bias=B1[:, a:a + 1],
                                         scale=A1[:, a:a + 1])
                    nc.vector.tensor_scalar(
                        z[:, 1:18, 1:33],
                        k[:, 0, 0:544].rearrange("p (y x) -> p y x", x=32),
                        sy1, ymin1, OP.mult, OP.add)
                    nc.scalar.activation(k[:, 0, 544:1024],
                                         out1[a][:, n, 544:1024], ACTF,
                                         bias=B1[:, a:a + 1],
                                         scale=A1[:, a:a + 1])
                    nc.vector.tensor_scalar(
                        z[:, 18:33, 1:33],
                        k[:, 0, 544:1024].rearrange("p (y x) -> p y x", x=32),
                        sy1, ymin1, OP.mult, OP.add)
                    zpads[(n, a)] = z
                return zpads[(n, a)]

            conv(range(NLOC), get_z, wl2, out1, sums2, mn2, mx2)
            A2, B2, sy2, ymin2 = bn_params(2, sums2, mn2, mx2, gb["g2"], gb["b2"])

            # ---------- final: out = qx + dequant(round(A2*conv2+B2)) ----------
            # round on ACT; dequant in place in k (first chunks DVE for
            # latency, rest on Pool for bandwidth); residual add on DVE into
            # v slots carved from the dead x16 buffer; 2-queue DMAs out.
            # round on ACT; dequant in place in k (first chunks DVE for
            # latency, Pool in the middle); residual add on DVE into v slots
            # carved from the dead x16 buffer; 2-queue DMAs out.
            xv = _x16f
            c = 0
            voff = 0
            CHUNKS_T = [(0, 1), (1, 1), (2, 2), (4, 2), (6, 1), (7, 1)]
            for n0, cnt in CHUNKS_T:
                for a in (0, 1):
                    k = kp.tile([P, cnt, HW], I32, tag=f"k{cnt}",
                                bufs=(4 if cnt == 1 else 3))
                    nc.scalar.activation(k[:, :, :], out1[a][:, n0:n0 + cnt, :],
                                         ACTF, bias=B2[:, a:a + 1],
                                         scale=A2[:, a:a + 1])
                    u = k.bitcast(F32)
                    ueng = nc.vector if (c < 2 or c >= 10) else nc.gpsimd
                    ueng.tensor_scalar(u[:, :, :], k[:, :, :],
                                       sy2, ymin2, OP.mult, OP.add)
                    if voff + cnt * HW > 8192:
                        voff = 0
                    v = xv[:, voff:voff + cnt * HW]
                    voff += cnt * HW
                    nc.vector.tensor_tensor(
                        v.rearrange("p (n y x) -> p n y x", y=32, x=32),
                        u.rearrange("p n (y x) -> p n y x", x=32),
                        qxpad[:, a * NLOC + n0:a * NLOC + n0 + cnt, 1:33, 1:33],
                        OP.add)
                    deng = nc.sync if a == 0 else nc.scalar
                    deng.dma_start(
                        out_d.ap()[n0:n0 + cnt, a * P:(a + 1) * P, :]
                        .rearrange("n c h -> c n h"),
                        v.rearrange("p (n h) -> p n h", n=cnt))
                    c += 1

    nc.compile()
    return nc


def kernel(**inputs):
    global _cached_nc
    from concourse import bass_utils

    x = np.ascontiguousarray(np.asarray(inputs["x"], dtype=np.float32)
                             .reshape(64, C, HW))
    w1 = np.asarray(inputs["w1"], dtype=np.float32).reshape(C, C, 9)
    w2 = np.asarray(inputs["w2"], dtype=np.float32).reshape(C, C, 9)
    w1t = np.ascontiguousarray(w1.transpose(1, 0, 2))
    w2t = np.ascontiguousarray(w2.transpose(1, 0, 2))
    g1 = np.ascontiguousarray(np.asarray(inputs["gamma1"], dtype=np.float32))
    b1 = np.ascontiguousarray(np.asarray(inputs["beta1"], dtype=np.float32))
    g2 = np.ascontiguousarray(np.asarray(inputs["gamma2"], dtype=np.float32))
    b2 = np.ascontiguousarray(np.asarray(inputs["beta2"], dtype=np.float32))

    if _cached_nc is None:
        _cached_nc = _build()
    nc = _cached_nc

    eye = np.eye(P, dtype=np.float16)
    in_maps = []
    for c in range(N_CORES):
        in_maps.append({
            "x": np.ascontiguousarray(x[c * NLOC:(c + 1) * NLOC]),
            "w1t": w1t, "w2t": w2t, "eye": eye,
            "gamma1": g1, "beta1": b1, "gamma2": g2, "beta2": b2,
        })
    res = bass_utils.run_bass_kernel_spmd(
        nc, in_maps, core_ids=list(range(N_CORES)))
    out = np.concatenate(
        [res.results[c]["out"].reshape(NLOC, C, 32, 32) for c in range(N_CORES)],
        axis=0)
    kernel.last_results = res
    return out



# revision 15
# speedup vs baseline: 1.0849x; 1.0849x over previous
"""Trainium2 Bass kernel for a quantized BasicBlock (QConv3x3 -> RangeNorm ->
QConv3x3 -> RangeNorm -> quantized residual add).

Sharding: data-parallel over batch (8 images per core across 8 cores);
weights replicated; per-tensor quantization min/max and per-channel
range-norm stats combined across cores with small collectives.

Per core (v2):
  - x uploaded fp16 (halves the input DMA) straight into the resident x16
    buffer in 16 half-image chunks on 2 queues. Per-chunk -min on DVE
    (flat reduces); max via a Pool TT-tree over chunks 0..11 (scratch in
    the not-yet-written out1 buffers) + flat DVE reduces for chunks 12..15.
    One packed partition_all_reduce + one AllReduce(max) of (-min, max).
  - w1/w2 uploaded fp16 in [ci, 9, co] layout; block min/max on DVE/Pool,
    round to int16 codes on Pool (RNE via dtype convert), dequant-relayout
    to fp16 on DVE (packed 2x), split per co-half so conv1 starts early.
  - qx pass: k = rne((x-xmin)/s) on ACT (fp16 -> int16), dequant to the
    zero-halo-free padded fp16 qxpad on DVE/Pool. Image 0 in halves so
    conv1's first tile starts after 17 interior rows.
  - conv3x3 = 18 accumulating PE matmuls per [co_block, half-image] PSUM
    tile; image-edge taps skip halo-only rows/cols (halo never read, so
    no halo memsets). Per-tile -min/max read directly from PSUM on
    DVE/Pool; sums via the ACT psum->SBUF copy accumulator.
  - RangeNorm stats: one AllGather of [128,6] per-channel (sum,-min,max)
    followed by local folds (sum-add / max) and a short two-engine param
    chain; bn output quantizer scale derived analytically.
  - z = quant(rangenorm(out1)) built in halves as int16 codes + fp16
    dequant; conv2 reuses out1.
  - final: out = qx + dequant(rne(A2*conv2 + B2)); round/dequant/add
    rotated across ACT/DVE/Pool, int16 codes dequantized in place, fp16
    result DMA'd out on 2 queues (fp16 download, upcast on host).
"""

import os
import numpy as np

N_CORES = 8
NLOC = 8            # images per core
C = 256
P = 128
HW = 1024           # 32*32
PAD = 34            # 32+2
EPS = 1e-8
QMAX = 255.0
N_TOTAL = 64 * 32 * 32          # range-norm n (global batch)
C_N = float(1.0 / np.sqrt(2.0 * np.log(N_TOTAL)))
NW = C * 9          # w block free size (per ci block)

_cached_nc = None


def _build(sim_single=False, no_collectives=False):
    """sim_single=True builds a 1-core variant with collectives replaced by
    a stand-in DMA — numerically wrong across cores but structurally
    identical, for TimelineSim cost-model analysis. no_collectives=True keeps
    8 cores but swaps collectives for local DMAs (timing A/B only)."""
    import concourse.bass as bass
    import concourse.mybir as mybir
    from concourse import bacc, tile
    import concourse.bass_isa as bass_isa

    dt = mybir.dt
    F32, F16, I16 = dt.float32, dt.float16, dt.int16
    AX = mybir.AxisListType.X
    AXY = mybir.AxisListType.XY
    OP = mybir.AluOpType
    ACTF = mybir.ActivationFunctionType.Identity
    RMAX = bass_isa.ReduceOp.max

    nc = bacc.Bacc("TRN2", target_bir_lowering=False, debug=False,
                   num_devices=(1 if sim_single else N_CORES))

    local_cc = sim_single or no_collectives

    def _flat(ap):
        names = "abcde"[:len(ap.shape)]
        if len(names) == 1:
            return ap
        spec = " ".join(names)
        return ap.rearrange(f"{spec} -> ({spec})")

    def allreduce(op, snd, rcv):
        if local_cc:
            nc.sync.dma_start(_flat(rcv)[None, :], _flat(snd)[None, :])
        else:
            nc.gpsimd.collective_compute(
                "AllReduce", op,
                replica_groups=[list(range(N_CORES))],
                ins=[snd.opt()], outs=[rcv.opt()])

    def allgather(snd, rcv, nelem):
        if local_cc:
            nc.sync.dma_start(_flat(rcv)[0:nelem][None, :], _flat(snd)[None, :])
        else:
            nc.gpsimd.collective_compute(
                "AllGather", mybir.AluOpType.bypass,
                replica_groups=[list(range(N_CORES))],
                ins=[snd.opt()], outs=[rcv.opt()])

    x_d = nc.dram_tensor("x", [NLOC, C, HW], F16, kind="ExternalInput")
    w1_d = nc.dram_tensor("w1t", [C, 9, C], F16, kind="ExternalInput")
    w2_d = nc.dram_tensor("w2t", [C, 9, C], F16, kind="ExternalInput")
    g1_d = nc.dram_tensor("gamma1", [C], F32, kind="ExternalInput")
    b1_d = nc.dram_tensor("beta1", [C], F32, kind="ExternalInput")
    g2_d = nc.dram_tensor("gamma2", [C], F32, kind="ExternalInput")
    b2_d = nc.dram_tensor("beta2", [C], F32, kind="ExternalInput")
    out_d = nc.dram_tensor("out", [NLOC, C, HW], F16, kind="ExternalOutput")

    with tile.TileContext(nc) as tc:
        with tc.tile_pool(name="consts", bufs=1) as cp, \
             tc.tile_pool(name="dram", bufs=1, space="DRAM") as dp, \
             tc.tile_pool(name="psum", bufs=8, space="PSUM") as pp, \
             tc.tile_pool(name="ktmp", bufs=3) as kp:

            # ---------- persistent tiles ----------
            qxpad = cp.tile([P, 2 * NLOC, PAD, PAD], F16, tag="qxpad")
            x16 = cp.tile([P, 2, NLOC, HW], F16, tag="x16", name="x16")
            out1 = [cp.tile([P, NLOC, HW], F16, tag=f"out1_{a}",
                            name=f"out1_{a}") for a in (0, 1)]
            wl1 = [cp.tile([P, 9, C], F16, tag=f"wl1_{a}", name=f"wl1_{a}")
                   for a in (0, 1)]
            wl2 = [cp.tile([P, 9, C], F16, tag=f"wl2_{a}", name=f"wl2_{a}")
                   for a in (0, 1)]
            zb = [cp.tile([P, PAD, PAD], F16, tag=f"zb_{i}", name=f"zb_{i}")
                  for i in range(2)]
            wraw = [cp.tile([P, NW], F16, tag=f"wraw_{a}", name=f"wraw_{a}")
                    for a in (0, 1)]
            kw = [cp.tile([P, 9, C], I16, tag=f"kw_{a}", name=f"kw_{a}")
                  for a in (0, 1)]

            # =====================================================
            # x stream: 16 half-image fp16 chunks straight into x16.
            # min: DVE TT-tree (fp16 2x) with a fused ttr top. max: Pool
            # XYZWC scalar reduces for chunks 0..11 + a DVE TT-tree for
            # chunks 12..15. w1 block min/max (DVE ttr trees) ride the
            # same single AllReduce(max) of [-xmin, xmax, -w1min, w1max].
            # Tree scratch lands in the idle out1 buffers.
            # =====================================================
            ps4 = cp.tile([P, 4], F32, tag="ps4")    # -min, max, -w1n, w1x
            xg = cp.tile([1, 8], F32, tag="xg")      # Pool chunk maxima 0..7
            o1v = [out1[i].rearrange("p n s -> p (n s)") for i in (0, 1)]

            def mslot(i):
                """[P,1024] tree scratch: slots 0-7 in out1[0], 8+ in
                out1[1] (both unwritten until conv1's psum copies)."""
                return o1v[i // 8][:, (i % 8) * HW:((i % 8) + 1) * HW]

            def xch(c):
                return x16[:, c % 2, c // 2, :]

            for j in range(NLOC):
                for a in (0, 1):
                    c = 2 * j + a
                    eng = nc.sync if a == 0 else nc.scalar
                    eng.dma_start(x16[:, a, j, :], x_d.ap()[j, a * P:(a + 1) * P, :])
                    if c <= 7:
                        nc.gpsimd.tensor_reduce(
                            xg[:, c:c + 1], xch(c),
                            mybir.AxisListType.XYZWC, OP.max)
                # min tree leaf for the image pair
                nc.vector.tensor_tensor(mslot(j), xch(2 * j), xch(2 * j + 1),
                                        OP.min)
                if j % 2 == 1:
                    nc.vector.tensor_tensor(mslot(8 + j // 2), mslot(j - 1),
                                            mslot(j), OP.min)
                if j >= 4:
                    # max-side leaves for chunks 8..15 (consumed min-leaf
                    # slots 1,2 are free for reuse)
                    ms = (1, 2, 14, 15)[j - 4]
                    nc.vector.tensor_tensor(mslot(ms), xch(2 * j),
                                            xch(2 * j + 1), OP.max)
            # min tree top over the 4 uppers -> -min
            nc.vector.tensor_tensor(mslot(12), mslot(8), mslot(9), OP.min)
            nc.vector.tensor_tensor(mslot(13), mslot(10), mslot(11), OP.min)
            nc.vector.tensor_tensor(mslot(0), mslot(12), mslot(13), OP.min)
            nc.vector.tensor_reduce(ps4[:, 0:1], mslot(0), AX, OP.min,
                                    negate=True)
            # max tree top over chunks 8..15
            nc.vector.tensor_tensor(mslot(3), mslot(1), mslot(2), OP.max)
            nc.vector.tensor_tensor(mslot(5), mslot(14), mslot(15), OP.max)
            nc.vector.tensor_tensor(mslot(4), mslot(3), mslot(5), OP.max)
            nc.vector.tensor_reduce(ps4[:, 1:2], mslot(4), AX, OP.max)

            # x collective fires as soon as local x stats land; the w1
            # pipeline overlaps its round-trip latency.
            snd_x = dp.tile([2], F32, tag="snd_x")
            rcv_x = dp.tile([2], F32, tag="rcv_x",
                            addr_space=("Local" if local_cc else "Shared"))
            gxp = cp.tile([P, 2], F32, tag="gxp")
            xgf = cp.tile([1, 1], F32, tag="xgf")
            nc.vector.tensor_reduce(xgf[:, :], xg[:, :], AX, OP.max)
            nc.gpsimd.partition_all_reduce(gxp[:, 0:2], ps4[:, 0:2], P, RMAX)
            nc.vector.tensor_tensor(gxp[0:1, 1:2], gxp[0:1, 1:2],
                                    xgf[0:1, :], OP.max)
            nc.sync.dma_start(snd_x[None, :], gxp[0:1, 0:2])
            allreduce(OP.max, snd_x, rcv_x)
            gx = cp.tile([P, 2], F32, tag="gx")
            nc.scalar.dma_start(gx[:, :], rcv_x[None, :].broadcast_to([P, 2]))

            # =====================================================
            # w1: fp16 [ci, 9, co] in 4 pieces per ci block on both queues
            # (behind x on the bus); block stats DVE/Pool; round to int16 on
            # Pool; dequant-relayout on DVE split per co-half so conv1's
            # first tiles aren't gated on the whole weight pipeline.
            # =====================================================
            def w_load(w_dram, q2=None):
                # q2: engine for the odd DMA queue. w1 streams while ACT is
                # idle (scalar queue); w2 must stay off the ACT SEQ so its
                # dispatch never blocks the latency-critical k rounds.
                q2 = q2 or nc.scalar
                for a in (0, 1):
                    src = w_dram.ap()[a * P:(a + 1) * P, :, :].rearrange(
                        "p q c -> p (q c)")
                    step = NW // 4
                    for piece in range(4):
                        lo = piece * step
                        eng = nc.sync if (piece + a) % 2 == 0 else q2
                        eng.dma_start(wraw[a][:, lo:lo + step],
                                      src[:, lo:lo + step])

            def w_stats(idx):
                wst = cp.tile([P, 4], F32, tag=f"wst{idx}", name=f"wst{idx}")
                # cols: [-min a0, -min a1, max a0, max a1]
                nc.vector.tensor_reduce(wst[:, 0:1], wraw[0][:, :], AX,
                                        OP.min, negate=True)
                nc.vector.tensor_reduce(wst[:, 2:3], wraw[0][:, :], AX, OP.max)
                nc.vector.tensor_reduce(wst[:, 1:2], wraw[1][:, :], AX,
                                        OP.min, negate=True)
                nc.vector.tensor_reduce(wst[:, 3:4], wraw[1][:, :], AX, OP.max)
                return wst

            def w_scalars(idx, wst):
                def s1(tag):
                    return cp.tile([P, 1], F32, tag=f"{tag}{idx}",
                                   name=f"{tag}{idx}")
                pnx = cp.tile([P, 2], F32, tag=f"wpnx{idx}", name=f"wpnx{idx}")
                nc.vector.tensor_reduce(pnx[:, 0:1], wst[:, 0:2], AX, OP.max)
                nc.vector.tensor_reduce(pnx[:, 1:2], wst[:, 2:4], AX, OP.max)
                gw = cp.tile([P, 2], F32, tag=f"wgw{idx}", name=f"wgw{idx}")
                nc.gpsimd.partition_all_reduce(gw[:, 0:2], pnx[:, 0:2], P, RMAX)
                rng = s1("wrng")
                nc.vector.tensor_tensor(rng[:], gw[:, 1:2], gw[:, 0:1], OP.add)
                s = s1("ws_")
                nc.vector.tensor_scalar(s[:], rng[:], 1.0 / QMAX, EPS,
                                        OP.mult, OP.max)
                inv = s1("winv")
                nc.vector.reciprocal(inv[:], s[:])
                bias = s1("wbias")
                nc.vector.tensor_tensor(bias[:], gw[:, 0:1], inv[:], OP.mult)
                gmn = s1("wgmn")
                nc.vector.tensor_scalar(gmn[:], gw[:, 0:1], -1.0, None, OP.mult)
                return s, inv, bias, gmn

            def w_round_relayout(wl, inv, bias, s, gmn, quarters,
                                 fast=False):
                # round to int16 codes + dequant-relayout to fp16 per
                # (co-half, ci-block). fast: cb0 rounds on DVE (2x int16)
                # for the conv1-critical quarters; otherwise Pool.
                wrv = [wraw[a].rearrange("p (q c) -> p q c", c=C)
                       for a in (0, 1)]
                for cb, a in quarters:
                    cs = slice(cb * P, (cb + 1) * P)
                    reng = nc.vector if (fast and cb == 0) else nc.gpsimd
                    reng.tensor_scalar(
                        kw[a][:, :, cs], wrv[a][:, :, cs],
                        inv[:, 0:1], bias[:, 0:1], OP.mult, OP.add)
                    nc.vector.tensor_scalar(
                        wl[a][:, :, cs], kw[a][:, :, cs],
                        s[:, 0:1], gmn[:, 0:1], OP.mult, OP.add)

            # w1 stats: weights replicated, so local stats are already
            # global — no collective, just a partition all-reduce. These
            # DVE trees overlap the x collective's round-trip latency.
            w_load(w1_d)
            wt = [cp.tile([P, NW // 2], F16, tag=f"wt{i}", name=f"wt{i}")
                  for i in range(3)]
            for side, (op, col) in enumerate(((OP.min, 2), (OP.max, 3))):
                nc.vector.tensor_tensor(wt[0][:, :], wraw[0][:, 0:NW // 2],
                                        wraw[0][:, NW // 2:NW], op)
                nc.vector.tensor_tensor(wt[1][:, :], wraw[1][:, 0:NW // 2],
                                        wraw[1][:, NW // 2:NW], op)
                nc.vector.tensor_tensor(wt[2][:, :], wt[0][:, :],
                                        wt[1][:, :], op)
                nc.vector.tensor_reduce(ps4[:, col:col + 1], wt[2][:, :], AX,
                                        op, negate=(op == OP.min))
            gw = cp.tile([P, 2], F32, tag="gw")
            nc.gpsimd.partition_all_reduce(gw[:, 0:2], ps4[:, 2:4], P, RMAX)

            # ---------- w1 + x quant scalars ----------
            def qscalars(pref, nmn, mx):
                rng = cp.tile([P, 1], F32, tag=f"{pref}rng")
                nc.vector.tensor_tensor(rng[:], mx, nmn, OP.add)
                s = cp.tile([P, 1], F32, tag=f"{pref}s")
                nc.vector.tensor_scalar(s[:], rng[:], 1.0 / QMAX, EPS,
                                        OP.mult, OP.max)
                inv = cp.tile([P, 1], F32, tag=f"{pref}inv")
                nc.vector.reciprocal(inv[:], s[:])
                bias = cp.tile([P, 1], F32, tag=f"{pref}bias")
                nc.vector.tensor_tensor(bias[:], nmn, inv[:], OP.mult)
                mn = cp.tile([P, 1], F32, tag=f"{pref}mn")
                nc.vector.tensor_scalar(mn[:], nmn, -1.0, None, OP.mult)
                return s, inv, bias, mn

            w1s, w1inv, w1bias, w1gmn = qscalars("w1", gw[:, 0:1], gw[:, 1:2])
            sx, invsx, biasx, xminv = qscalars("x", gx[:, 0:1], gx[:, 1:2])

            # =====================================================
            # qx pass: k = rne((x-xmin)/s) on ACT (int16), dequant into the
            # padded fp16 qxpad. Image 0 in halves (subtile deps) so conv1
            # starts after the top 17 interior rows. w1 quarters interleave
            # so DVE alternates between wl1 prep and the first qx deqs.
            # =====================================================
            def emit_chunk(n0, cnt):
                for a in (0, 1):
                    k = kp.tile([P, cnt, HW], I16, tag=f"k{cnt}",
                                bufs=(4 if cnt == 1 else 3))
                    if n0 == 0:
                        for lo, hi, r0, r1 in ((0, 544, 1, 18),
                                               (544, 1024, 18, 33)):
                            nc.scalar.activation(
                                k[:, 0, lo:hi], x16[:, a, 0, lo:hi], ACTF,
                                bias=biasx[:, 0:1], scale=invsx[:, 0:1])
                            nc.vector.tensor_scalar(
                                qxpad[:, a * NLOC, r0:r1, 1:33],
                                k[:, 0, lo:hi].rearrange(
                                    "p (y x) -> p y x", x=32),
                                sx[:, 0:1], xminv[:, 0:1], OP.mult, OP.add)
                        continue
                    nc.scalar.activation(k[:, :, :], x16[:, a, n0:n0 + cnt, :],
                                         ACTF, bias=biasx[:, 0:1],
                                         scale=invsx[:, 0:1])
                    deng = nc.vector if n0 <= 2 else nc.gpsimd
                    deng.tensor_scalar(
                        qxpad[:, a * NLOC + n0:a * NLOC + n0 + cnt, 1:33, 1:33],
                        k.rearrange("p n (y x) -> p n y x", x=32),
                        sx[:, 0:1], xminv[:, 0:1], OP.mult, OP.add)

            w_round_relayout(wl1, w1inv, w1bias, w1s, w1gmn, [(0, 0)],
                             fast=True)
            emit_chunk(0, 1)
            w_round_relayout(wl1, w1inv, w1bias, w1s, w1gmn, [(0, 1)],
                             fast=True)
            emit_chunk(1, 1)
            w_round_relayout(wl1, w1inv, w1bias, w1s, w1gmn,
                             [(1, 0), (1, 1)], fast=True)
            emit_chunk(2, 2)
            emit_chunk(4, 2)
            emit_chunk(6, 2)

            # =====================================================
            # conv helper: 18 matmuls per [co_block, half] PSUM tile;
            # -min/max stats straight from PSUM (DVE/Pool), sums via the
            # ACT copy accumulator.
            # =====================================================
            def conv(ns, in_pad_at, wl, outt, sums, mnt, mxt):
                for n in ns:
                    for cb in (0, 1):
                        for half in (0, 1):
                            ps = pp.tile([P, 512], F32, tag="ps")
                            i = 0
                            for a in (0, 1):
                                src = in_pad_at(n, a)
                                for ky in (1, 0, 2):
                                    r0, o0 = half * 16 + ky, 0
                                    rows = 16
                                    if ky == 0 and half == 0:
                                        r0, o0, rows = 1, 32, 15
                                    elif ky == 2 and half == 1:
                                        rows = 15
                                    for kx in (1, 0, 2):
                                        c0, x0, cols = kx, 0, 32
                                        if kx == 0:
                                            c0, x0, cols = 1, 1, 31
                                        elif kx == 2:
                                            cols = 31
                                        rhs = src[:, r0:r0 + rows,
                                                  c0:c0 + cols]
                                        pv = ps.rearrange(
                                            "p (y x) -> p y x", x=32)
                                        out = pv[:, o0 // 32:o0 // 32 + rows,
                                                 x0:x0 + cols]
                                        nc.tensor.matmul(
                                            out,
                                            wl[a][:, ky * 3 + kx,
                                                  cb * P:(cb + 1) * P],
                                            rhs, start=(i == 0), stop=(i == 17))
                                        i += 1
                            h = n * 2 + half
                            nc.scalar.activation(
                                outt[cb][:, n, half * 512:(half + 1) * 512],
                                ps[:], ACTF,
                                accum_out=sums[:, cb, h:h + 1])
                            nc.vector.tensor_reduce(
                                mnt[:, cb, h:h + 1], ps[:], AX, OP.min,
                                negate=True)
                            nc.vector.tensor_reduce(
                                mxt[:, cb, h:h + 1], ps[:], AX, OP.max)

            # =====================================================
            # range-norm stats: single AllGather of [P,6] per-channel
            # (sum, -min, max) + local folds -> fused affine params
            # =====================================================
            def bn_params(idx, sums, mnt, mxt, gt, bt):
                pk = cp.tile([P, 6], F32, tag=f"pk{idx}", name=f"pk{idx}")
                for cb in (0, 1):
                    nc.vector.tensor_reduce(pk[:, cb:cb + 1], sums[:, cb, :],
                                            AX, OP.add)
                    nc.vector.tensor_reduce(pk[:, 2 + cb:3 + cb],
                                            mnt[:, cb, :], AX, OP.max)
                    nc.vector.tensor_reduce(pk[:, 4 + cb:5 + cb],
                                            mxt[:, cb, :], AX, OP.max)
                snd_s = dp.tile([P, 2], F32, tag=f"snds{idx}",
                                name=f"snds{idx}")
                snd_m = dp.tile([P, 4], F32, tag=f"sndm{idx}",
                                name=f"sndm{idx}")
                rcv_s = dp.tile([P, 2], F32, tag=f"rcvs{idx}",
                                name=f"rcvs{idx}",
                                addr_space=("Local" if local_cc else "Shared"))
                rcv_m = dp.tile([P, 4], F32, tag=f"rcvm{idx}",
                                name=f"rcvm{idx}",
                                addr_space=("Local" if local_cc else "Shared"))
                nc.sync.dma_start(snd_s[:, :], pk[:, 0:2])
                nc.gpsimd.dma_start(snd_m[:, :], pk[:, 2:6])
                allreduce(OP.add, snd_s, rcv_s)
                allreduce(OP.max, snd_m, rcv_m)
                ssum = cp.tile([P, 2], F32, tag=f"ssum{idx}", name=f"ssum{idx}")
                stm4 = cp.tile([P, 4], F32, tag=f"stm4{idx}", name=f"stm4{idx}")
                nc.gpsimd.dma_start(ssum[:, :], rcv_s[:, :])
                nc.sync.dma_start(stm4[:, :], rcv_m[:, :])

                def t2(tag):
                    return cp.tile([P, 2], F32, tag=f"{tag}{idx}",
                                   name=f"{tag}{idx}")

                # params: a_ = g/(C_N*rng+EPS); b_ = b - a_*mean
                mean = t2("mean")
                nc.vector.tensor_scalar(mean[:], ssum[:, :], 1.0 / N_TOTAL,
                                        None, OP.mult)
                rng = t2("rng")
                nc.vector.tensor_tensor(rng[:], stm4[:, 2:4], stm4[:, 0:2],
                                        OP.add)
                sc = t2("sc")
                nc.gpsimd.tensor_scalar(sc[:], rng[:], C_N, EPS, OP.mult,
                                        OP.add)
                inv = t2("inv")
                nc.vector.reciprocal(inv[:], sc[:])
                a_ = t2("a_")
                nc.vector.tensor_tensor(a_[:], gt[:], inv[:], OP.mult)
                am = t2("am")
                nc.vector.tensor_tensor(am[:], a_[:], mean[:], OP.mult)
                b_ = t2("b_")
                nc.vector.tensor_tensor(b_[:], bt[:], am[:], OP.subtract)
                # per-channel output range: lo = a_*min+b_ = b_ - a_*(-min)
                t1_ = t2("t1_")
                nc.vector.tensor_tensor(t1_[:], a_[:], stm4[:, 0:2], OP.mult)
                lo = t2("lo")
                nc.vector.tensor_tensor(lo[:], b_[:], t1_[:], OP.subtract)
                t2_ = t2("t2_")
                nc.vector.tensor_tensor(t2_[:], a_[:], stm4[:, 2:4], OP.mult)
                hi = t2("hi")
                nc.vector.tensor_tensor(hi[:], t2_[:], b_[:], OP.add)
                lo2 = t2("lo2")
                hi2 = t2("hi2")
                nc.vector.tensor_tensor(lo2[:], lo[:], hi[:], OP.min)
                nc.vector.tensor_tensor(hi2[:], lo[:], hi[:], OP.max)
                pq = t2("pq")
                nc.vector.tensor_reduce(pq[:, 0:1], lo2[:], AX, OP.min,
                                        negate=True)
                nc.vector.tensor_reduce(pq[:, 1:2], hi2[:], AX, OP.max)
                nh = t2("nh")
                nc.gpsimd.partition_all_reduce(nh[:, 0:2], pq[:, 0:2], P, RMAX)

                def y1(tag):
                    return cp.tile([P, 1], F32, tag=f"{tag}{idx}",
                                   name=f"{tag}{idx}")
                ymin = y1("ymin")
                nc.vector.tensor_scalar(ymin[:], nh[:, 0:1], -1.0, None,
                                        OP.mult)
                rngy = y1("rngy")
                nc.vector.tensor_tensor(rngy[:], nh[:, 1:2], nh[:, 0:1],
                                        OP.add)
                sy = y1("sy")
                nc.gpsimd.tensor_scalar(sy[:], rngy[:], 1.0 / QMAX, EPS,
                                        OP.mult, OP.max)
                invsy = y1("invsy")
                nc.vector.reciprocal(invsy[:], sy[:])
                A = t2("A")
                nc.vector.tensor_scalar(A[:], a_[:], invsy[:, 0:1], None,
                                        OP.mult)
                B = t2("B")
                nc.gpsimd.tensor_scalar(B[:], b_[:], ymin[:, 0:1], None,
                                        OP.subtract)
                nc.gpsimd.tensor_scalar(B[:], B[:], invsy[:, 0:1], None,
                                        OP.mult)
                return A, B, sy[:, 0:1], ymin[:, 0:1]

            # ---------- conv1 (w2 prep + gamma/beta loads after the first
            # two images so they run in conv1's engine-idle time) ----------
            sums1 = cp.tile([P, 2, 16], F32, tag="sums1")
            mn1 = cp.tile([P, 2, 16], F32, tag="mn1")
            mx1 = cp.tile([P, 2, 16], F32, tag="mx1")
            nc.vector.memset(sums1[:, :, :], 0.0)
            qx_at = lambda n, a: qxpad[:, a * NLOC + n, :, :]
            conv(range(0, 2), qx_at, wl1, out1, sums1, mn1, mx1)

            w_load(w2_d, q2=nc.gpsimd)
            wst2 = w_stats(2)
            w2s, w2inv, w2bias, w2gmn = w_scalars(2, wst2)
            w_round_relayout(wl2, w2inv, w2bias, w2s, w2gmn,
                             [(0, 0), (0, 1), (1, 0), (1, 1)])
            gb = {}
            for i, (nm, d) in enumerate(
                    (("g1", g1_d), ("b1", b1_d), ("g2", g2_d), ("b2", b2_d))):
                t = cp.tile([P, 2], F32, tag=f"gb_{nm}", name=f"gb_{nm}")
                nc.sync.dma_start(t[:], d.ap().rearrange("(a p) -> p a", p=P))
                gb[nm] = t

            conv(range(2, NLOC), qx_at, wl1, out1, sums1, mn1, mx1)
            A1, B1, sy1, ymin1 = bn_params(1, sums1, mn1, mx1, gb["g1"],
                                           gb["b1"])

            # ---------- z = quant(rangenorm(out1)); conv2 into out1 ----------
            sums2 = cp.tile([P, 2, 16], F32, tag="sums2")
            mn2 = cp.tile([P, 2, 16], F32, tag="mn2")
            mx2 = cp.tile([P, 2, 16], F32, tag="mx2")
            nc.vector.memset(sums2[:, :, :], 0.0)
            zpads = {}

            def get_z(n, a):
                if (n, a) not in zpads:
                    z = zb[(2 * n + a) % 2]
                    k = kp.tile([P, 1, HW], I16, tag="k1", bufs=4)
                    nc.scalar.activation(k[:, 0, 0:544], out1[a][:, n, 0:544],
                                         ACTF, bias=B1[:, a:a + 1],
                                         scale=A1[:, a:a + 1])
                    nc.vector.tensor_scalar(
                        z[:, 1:18, 1:33],
                        k[:, 0, 0:544].rearrange("p (y x) -> p y x", x=32),
                        sy1, ymin1, OP.mult, OP.add)
                    nc.scalar.activation(k[:, 0, 544:1024],
                                         out1[a][:, n, 544:1024], ACTF,
                                         bias=B1[:, a:a + 1],
                                         scale=A1[:, a:a + 1])
                    nc.vector.tensor_scalar(
                        z[:, 18:33, 1:33],
                        k[:, 0, 544:1024].rearrange("p (y x) -> p y x", x=32),
                        sy1, ymin1, OP.mult, OP.add)
                    zpads[(n, a)] = z
                return zpads[(n, a)]

            conv(range(NLOC), get_z, wl2, out1, sums2, mn2, mx2)
            A2, B2, sy2, ymin2 = bn_params(2, sums2, mn2, mx2, gb["g2"],
                                           gb["b2"])

            # ---------- final: out = qx + dequant(rne(A2*conv2+B2)) ----------
            # round (ACT/Pool), dequant in place over the int16 codes
            # (DVE/Pool), residual add (DVE/Pool) into v slots carved from
            # the dead x16 buffer, fp16 out on 2 DMA queues.
            xv = x16.rearrange("p a n s -> p (a n s)")
            voff = 0
            c = 0
            CHUNKS_T = [(0, 1), (1, 1), (2, 2), (4, 2), (6, 1), (7, 1)]
            for n0, cnt in CHUNKS_T:
                for a in (0, 1):
                    k = kp.tile([P, cnt, HW], I16, tag=f"k{cnt}",
                                bufs=(4 if cnt == 1 else 3))
                    # round: ACT mostly; one 2-image unit on Pool
                    if c == 5:
                        nc.gpsimd.tensor_scalar(k[:, :, :],
                                                out1[a][:, n0:n0 + cnt, :],
                                                A2[:, a:a + 1], B2[:, a:a + 1],
                                                OP.mult, OP.add)
                    else:
                        nc.scalar.activation(k[:, :, :],
                                             out1[a][:, n0:n0 + cnt, :],
                                             ACTF, bias=B2[:, a:a + 1],
                                             scale=A2[:, a:a + 1])
                    u = k.bitcast(F16)
                    ueng = nc.gpsimd if c in (3, 4, 6, 7) else nc.vector
                    ueng.tensor_scalar(u[:, :, :], k[:, :, :],
                                       sy2, ymin2, OP.mult, OP.add)
                    if voff + cnt * HW > 16384:
                        voff = 0
                    v = xv[:, voff:voff + cnt * HW]
                    voff += cnt * HW
                    aeng = nc.vector
                    aeng.tensor_tensor(
                        v.rearrange("p (n y x) -> p n y x", y=32, x=32),
                        u.rearrange("p n (y x) -> p n y x", x=32),
                        qxpad[:, a * NLOC + n0:a * NLOC + n0 + cnt, 1:33, 1:33],
                        OP.add)
                    nc.sync.dma_start(
                        out_d.ap()[n0:n0 + cnt, a * P:(a + 1) * P, :]
                        .rearrange("n c h -> c n h"),
                        v.rearrange("p (n h) -> p n h", n=cnt))
                    c += 1

    nc.compile()
    return nc


def kernel(**inputs):
    global _cached_nc
    from concourse import bass_utils

    x = np.ascontiguousarray(
        np.asarray(inputs["x"], dtype=np.float32).reshape(64, C, HW)
        .astype(np.float16))
    w1 = np.asarray(inputs["w1"], dtype=np.float32).reshape(C, C, 9)
    w2 = np.asarray(inputs["w2"], dtype=np.float32).reshape(C, C, 9)
    # [co, ci, q] -> [ci, q, co], fp16
    w1t = np.ascontiguousarray(w1.transpose(1, 2, 0).astype(np.float16))
    w2t = np.ascontiguousarray(w2.transpose(1, 2, 0).astype(np.float16))
    g1 = np.ascontiguousarray(np.asarray(inputs["gamma1"], dtype=np.float32))
    b1 = np.ascontiguousarray(np.asarray(inputs["beta1"], dtype=np.float32))
    g2 = np.ascontiguousarray(np.asarray(inputs["gamma2"], dtype=np.float32))
    b2 = np.ascontiguousarray(np.asarray(inputs["beta2"], dtype=np.float32))

    if _cached_nc is None:
        _cached_nc = _build()
    nc = _cached_nc

    in_maps = []
    for cid in range(N_CORES):
        in_maps.append({
            "x": np.ascontiguousarray(x[cid * NLOC:(cid + 1) * NLOC]),
            "w1t": w1t, "w2t": w2t,
            "gamma1": g1, "beta1": b1, "gamma2": g2, "beta2": b2,
        })
    res = bass_utils.run_bass_kernel_spmd(
        nc, in_maps, core_ids=list(range(N_CORES)))
    out = np.concatenate(
        [res.results[cid]["out"].reshape(NLOC, C, 32, 32).astype(np.float32)
         for cid in range(N_CORES)],
        axis=0)
    kernel.last_results = res
    return out


# revision 34
# speedup vs baseline: 1.1293x; 1.0409x over previous
"""Trainium2 Bass kernel for a quantized BasicBlock (QConv3x3 -> RangeNorm ->
QConv3x3 -> RangeNorm -> quantized residual add).

Sharding: data-parallel over batch (8 images per core across 8 cores);
weights replicated; per-tensor quantization min/max and per-channel
range-norm stats combined across cores with small collectives.

Per core (v2):
  - x uploaded fp16 (halves the input DMA) straight into the resident x16
    buffer in 16 half-image chunks on 2 queues. Per-chunk -min on DVE
    (flat reduces); max via a Pool TT-tree over chunks 0..11 (scratch in
    the not-yet-written out1 buffers) + flat DVE reduces for chunks 12..15.
    One packed partition_all_reduce + one AllReduce(max) of (-min, max).
  - w1/w2 uploaded fp16 in [ci, 9, co] layout; block min/max on DVE/Pool,
    round to int16 codes on Pool (RNE via dtype convert), dequant-relayout
    to fp16 on DVE (packed 2x), split per co-half so conv1 starts early.
  - qx pass: k = rne((x-xmin)/s) on ACT (fp16 -> int16), dequant to the
    zero-halo-free padded fp16 qxpad on DVE/Pool. Image 0 in halves so
    conv1's first tile starts after 17 interior rows.
  - conv3x3 = 18 accumulating PE matmuls per [co_block, half-image] PSUM
    tile; image-edge taps skip halo-only rows/cols (halo never read, so
    no halo memsets). Per-tile -min/max read directly from PSUM on
    DVE/Pool; sums via the ACT psum->SBUF copy accumulator.
  - RangeNorm stats: one AllGather of [128,6] per-channel (sum,-min,max)
    followed by local folds (sum-add / max) and a short two-engine param
    chain; bn output quantizer scale derived analytically.
  - z = quant(rangenorm(out1)) built in halves as int16 codes + fp16
    dequant; conv2 reuses out1.
  - final: out = qx + dequant(rne(A2*conv2 + B2)); round/dequant/add
    rotated across ACT/DVE/Pool, int16 codes dequantized in place, fp16
    result DMA'd out on 2 queues (fp16 download, upcast on host).
"""

import os
import numpy as np

N_CORES = 8
NLOC = 8            # images per core
C = 256
P = 128
HW = 1024           # 32*32
PAD = 34            # 32+2
EPS = 1e-8
QMAX = 255.0
N_TOTAL = 64 * 32 * 32          # range-norm n (global batch)
C_N = float(1.0 / np.sqrt(2.0 * np.log(N_TOTAL)))
NW = C * 9          # w block free size (per ci block)

_cached_nc = None


def _build(sim_single=False, no_collectives=False):
    """sim_single=True builds a 1-core variant with collectives replaced by
    a stand-in DMA — numerically wrong across cores but structurally
    identical, for TimelineSim cost-model analysis. no_collectives=True keeps
    8 cores but swaps collectives for local DMAs (timing A/B only)."""
    import concourse.bass as bass
    import concourse.mybir as mybir
    from concourse import bacc, tile
    import concourse.bass_isa as bass_isa

    dt = mybir.dt
    F32, F16, I16 = dt.float32, dt.float16, dt.int16
    AX = mybir.AxisListType.X
    AXY = mybir.AxisListType.XY
    OP = mybir.AluOpType
    ACTF = mybir.ActivationFunctionType.Identity
    RMAX = bass_isa.ReduceOp.max

    nc = bacc.Bacc("TRN2", target_bir_lowering=False, debug=False,
                   num_devices=(1 if sim_single else N_CORES))

    local_cc = sim_single or no_collectives

    def _flat(ap):
        names = "abcde"[:len(ap.shape)]
        if len(names) == 1:
            return ap
        spec = " ".join(names)
        return ap.rearrange(f"{spec} -> ({spec})")

    def allreduce(op, snd, rcv):
        if local_cc:
            nc.sync.dma_start(_flat(rcv)[None, :], _flat(snd)[None, :])
        else:
            nc.gpsimd.collective_compute(
                "AllReduce", op,
                replica_groups=[list(range(N_CORES))],
                ins=[snd.opt()], outs=[rcv.opt()])

    def allgather(snd, rcv, nelem):
        if local_cc:
            nc.sync.dma_start(_flat(rcv)[0:nelem][None, :], _flat(snd)[None, :])
        else:
            nc.gpsimd.collective_compute(
                "AllGather", mybir.AluOpType.bypass,
                replica_groups=[list(range(N_CORES))],
                ins=[snd.opt()], outs=[rcv.opt()])

    x_d = nc.dram_tensor("x", [NLOC, C, HW], F16, kind="ExternalInput")
    w1_d = nc.dram_tensor("w1t", [C, 9, C], F16, kind="ExternalInput")
    w2_d = nc.dram_tensor("w2t", [C, 9, C], F16, kind="ExternalInput")
    g1_d = nc.dram_tensor("gamma1", [C], F32, kind="ExternalInput")
    b1_d = nc.dram_tensor("beta1", [C], F32, kind="ExternalInput")
    g2_d = nc.dram_tensor("gamma2", [C], F32, kind="ExternalInput")
    b2_d = nc.dram_tensor("beta2", [C], F32, kind="ExternalInput")
    out_d = nc.dram_tensor("out", [NLOC, C, HW], F16, kind="ExternalOutput")

    with tile.TileContext(nc) as tc:
        with tc.tile_pool(name="consts", bufs=1) as cp, \
             tc.tile_pool(name="dram", bufs=1, space="DRAM") as dp, \
             tc.tile_pool(name="psum", bufs=8, space="PSUM") as pp, \
             tc.tile_pool(name="ktmp", bufs=3) as kp:

            # ---------- persistent tiles ----------
            qxpad = cp.tile([P, 2 * NLOC, PAD, PAD], F16, tag="qxpad")
            x16 = cp.tile([P, 2, NLOC, HW], F16, tag="x16", name="x16")
            out1 = [cp.tile([P, NLOC, HW], F16, tag=f"out1_{a}",
                            name=f"out1_{a}") for a in (0, 1)]
            wl1 = [cp.tile([P, 9, C], F16, tag=f"wl1_{a}", name=f"wl1_{a}")
                   for a in (0, 1)]
            wl2 = [cp.tile([P, 9, C], F16, tag=f"wl2_{a}", name=f"wl2_{a}")
                   for a in (0, 1)]
            zb = [cp.tile([P, PAD, PAD], F16, tag=f"zb_{i}", name=f"zb_{i}")
                  for i in range(2)]
            wraw = [cp.tile([P, NW], F16, tag=f"wraw_{a}", name=f"wraw_{a}")
                    for a in (0, 1)]
            kw = [cp.tile([P, 9, C], I16, tag=f"kw_{a}", name=f"kw_{a}")
                  for a in (0, 1)]

            # PE warmup: dummy matmuls keep the tensor engine's p-state
            # ramped through the startup and inter-conv stat barriers
            # (PE is otherwise idle there, so they cost nothing).
            wtile = cp.tile([P, 512], F16, tag="wtile")
            nc.vector.memset(wtile[:, :], 0.0)

            def warmup(n):
                # rotate through the conv psum banks (idle while PE idles)
                # so consecutive warmups overlap their WAW semaphores
                for _ in range(n):
                    pw = pp.tile([P, 512], F32, tag="ps", bufs=8)
                    nc.tensor.matmul(pw[:, :], wtile[:, 0:P], wtile[:, :],
                                     start=True, stop=True)

            warmup(144)

            # =====================================================
            # x stream: 16 half-image fp16 chunks straight into x16.
            # min: DVE TT-tree (fp16 2x) with a fused ttr top. max: Pool
            # XYZWC scalar reduces for chunks 0..11 + a DVE TT-tree for
            # chunks 12..15. w1 block min/max (DVE ttr trees) ride the
            # same single AllReduce(max) of [-xmin, xmax, -w1min, w1max].
            # Tree scratch lands in the idle out1 buffers.
            # =====================================================
            ps4 = cp.tile([P, 4], F32, tag="ps4")    # -min, max, -w1n, w1x
            xg = cp.tile([1, 8], F32, tag="xg")      # Pool chunk maxima 0..7
            o1v = [out1[i].rearrange("p n s -> p (n s)") for i in (0, 1)]

            def mslot(i):
                """[P,1024] tree scratch: slots 0-7 in out1[0], 8+ in
                out1[1] (both unwritten until conv1's psum copies)."""
                return o1v[i // 8][:, (i % 8) * HW:((i % 8) + 1) * HW]

            def xch(c):
                return x16[:, c % 2, c // 2, :]

            for j in range(NLOC):
                for a in (0, 1):
                    c = 2 * j + a
                    eng = nc.sync if a == 0 else nc.scalar
                    eng.dma_start(x16[:, a, j, :], x_d.ap()[j, a * P:(a + 1) * P, :])
                    if c <= 7:
                        nc.gpsimd.tensor_reduce(
                            xg[:, c:c + 1], xch(c),
                            mybir.AxisListType.XYZWC, OP.max)
                # min tree leaf for the image pair
                nc.vector.tensor_tensor(mslot(j), xch(2 * j), xch(2 * j + 1),
                                        OP.min)
                if j % 2 == 1:
                    nc.vector.tensor_tensor(mslot(8 + j // 2), mslot(j - 1),
                                            mslot(j), OP.min)
                if j >= 4:
                    # max-side leaves for chunks 8..15 (consumed min-leaf
                    # slots 1,2 are free for reuse)
                    ms = (1, 2, 14, 15)[j - 4]
                    nc.vector.tensor_tensor(mslot(ms), xch(2 * j),
                                            xch(2 * j + 1), OP.max)
            # min tree top over the 4 uppers -> -min
            nc.vector.tensor_tensor(mslot(12), mslot(8), mslot(9), OP.min)
            nc.vector.tensor_tensor(mslot(13), mslot(10), mslot(11), OP.min)
            nc.vector.tensor_tensor(mslot(0), mslot(12), mslot(13), OP.min)
            nc.vector.tensor_reduce(ps4[:, 0:1], mslot(0), AX, OP.min,
                                    negate=True)
            # max tree top over chunks 8..15
            nc.vector.tensor_tensor(mslot(3), mslot(1), mslot(2), OP.max)
            nc.vector.tensor_tensor(mslot(5), mslot(14), mslot(15), OP.max)
            nc.vector.tensor_tensor(mslot(4), mslot(3), mslot(5), OP.max)
            nc.vector.tensor_reduce(ps4[:, 1:2], mslot(4), AX, OP.max)

            # x collective fires as soon as local x stats land; the w1
            # pipeline overlaps its round-trip latency.
            snd_x = dp.tile([2], F32, tag="snd_x")
            rcv_x = dp.tile([2], F32, tag="rcv_x",
                            addr_space=("Local" if local_cc else "Shared"))
            gxp = cp.tile([P, 2], F32, tag="gxp")
            xgf = cp.tile([1, 1], F32, tag="xgf")
            nc.vector.tensor_reduce(xgf[:, :], xg[:, :], AX, OP.max)
            nc.gpsimd.partition_all_reduce(gxp[:, 0:2], ps4[:, 0:2], P, RMAX)
            nc.vector.tensor_tensor(gxp[0:1, 1:2], gxp[0:1, 1:2],
                                    xgf[0:1, :], OP.max)
            nc.sync.dma_start(snd_x[None, :], gxp[0:1, 0:2])
            allreduce(OP.max, snd_x, rcv_x)
            gx = cp.tile([P, 2], F32, tag="gx")
            nc.scalar.dma_start(gx[:, :], rcv_x[None, :].broadcast_to([P, 2]))

            # =====================================================
            # w1: fp16 [ci, 9, co] in 4 pieces per ci block on both queues
            # (behind x on the bus); block stats DVE/Pool; round to int16 on
            # Pool; dequant-relayout on DVE split per co-half so conv1's
            # first tiles aren't gated on the whole weight pipeline.
            # =====================================================
            def w_load(w_dram, q2=None):
                # q2: engine for the odd DMA queue. w1 streams while ACT is
                # idle (scalar queue); w2 must stay off the ACT SEQ so its
                # dispatch never blocks the latency-critical k rounds.
                q2 = q2 or nc.scalar
                for a in (0, 1):
                    src = w_dram.ap()[a * P:(a + 1) * P, :, :].rearrange(
                        "p q c -> p (q c)")
                    step = NW // 4
                    for piece in range(4):
                        lo = piece * step
                        eng = nc.sync if (piece + a) % 2 == 0 else q2
                        eng.dma_start(wraw[a][:, lo:lo + step],
                                      src[:, lo:lo + step])

            def w_stats(idx):
                wst = cp.tile([P, 4], F32, tag=f"wst{idx}", name=f"wst{idx}")
                # cols: [-min a0, -min a1, max a0, max a1]
                nc.vector.tensor_reduce(wst[:, 0:1], wraw[0][:, :], AX,
                                        OP.min, negate=True)
                nc.vector.tensor_reduce(wst[:, 2:3], wraw[0][:, :], AX, OP.max)
                nc.vector.tensor_reduce(wst[:, 1:2], wraw[1][:, :], AX,
                                        OP.min, negate=True)
                nc.vector.tensor_reduce(wst[:, 3:4], wraw[1][:, :], AX, OP.max)
                return wst

            def w_scalars(idx, wst):
                def s1(tag):
                    return cp.tile([P, 1], F32, tag=f"{tag}{idx}",
                                   name=f"{tag}{idx}")
                pnx = cp.tile([P, 2], F32, tag=f"wpnx{idx}", name=f"wpnx{idx}")
                nc.vector.tensor_reduce(pnx[:, 0:1], wst[:, 0:2], AX, OP.max)
                nc.vector.tensor_reduce(pnx[:, 1:2], wst[:, 2:4], AX, OP.max)
                gw = cp.tile([P, 2], F32, tag=f"wgw{idx}", name=f"wgw{idx}")
                nc.gpsimd.partition_all_reduce(gw[:, 0:2], pnx[:, 0:2], P, RMAX)
                rng = s1("wrng")
                nc.vector.tensor_tensor(rng[:], gw[:, 1:2], gw[:, 0:1], OP.add)
                s = s1("ws_")
                nc.vector.tensor_scalar(s[:], rng[:], 1.0 / QMAX, EPS,
                                        OP.mult, OP.max)
                inv = s1("winv")
                nc.vector.reciprocal(inv[:], s[:])
                bias = s1("wbias")
                nc.vector.tensor_tensor(bias[:], gw[:, 0:1], inv[:], OP.mult)
                gmn = s1("wgmn")
                nc.vector.tensor_scalar(gmn[:], gw[:, 0:1], -1.0, None, OP.mult)
                return s, inv, bias, gmn

            def w_round_relayout(wl, inv, bias, s, gmn, quarters,
                                 fast=False):
                # round to int16 codes + dequant-relayout to fp16 per
                # (co-half, ci-block). fast: cb0 rounds on DVE (2x int16)
                # for the conv1-critical quarters; otherwise Pool.
                wrv = [wraw[a].rearrange("p (q c) -> p q c", c=C)
                       for a in (0, 1)]
                for cb, a in quarters:
                    cs = slice(cb * P, (cb + 1) * P)
                    reng = nc.vector if (fast and cb == 0) else nc.gpsimd
                    reng.tensor_scalar(
                        kw[a][:, :, cs], wrv[a][:, :, cs],
                        inv[:, 0:1], bias[:, 0:1], OP.mult, OP.add)
                    nc.vector.tensor_scalar(
                        wl[a][:, :, cs], kw[a][:, :, cs],
                        s[:, 0:1], gmn[:, 0:1], OP.mult, OP.add)

            # w1 stats: weights replicated, so local stats are already
            # global — no collective, just a partition all-reduce. These
            # DVE trees overlap the x collective's round-trip latency.
            w_load(w1_d)
            wt = [cp.tile([P, NW // 2], F16, tag=f"wt{i}", name=f"wt{i}")
                  for i in range(3)]
            for side, (op, col) in enumerate(((OP.min, 2), (OP.max, 3))):
                nc.vector.tensor_tensor(wt[0][:, :], wraw[0][:, 0:NW // 2],
                                        wraw[0][:, NW // 2:NW], op)
                nc.vector.tensor_tensor(wt[1][:, :], wraw[1][:, 0:NW // 2],
                                        wraw[1][:, NW // 2:NW], op)
                nc.vector.tensor_tensor(wt[2][:, :], wt[0][:, :],
                                        wt[1][:, :], op)
                nc.vector.tensor_reduce(ps4[:, col:col + 1], wt[2][:, :], AX,
                                        op, negate=(op == OP.min))
            gw = cp.tile([P, 2], F32, tag="gw")
            nc.gpsimd.partition_all_reduce(gw[:, 0:2], ps4[:, 2:4], P, RMAX)

            # ---------- w1 + x quant scalars ----------
            def qscalars(pref, nmn, mx):
                rng = cp.tile([P, 1], F32, tag=f"{pref}rng")
                nc.vector.tensor_tensor(rng[:], mx, nmn, OP.add)
                s = cp.tile([P, 1], F32, tag=f"{pref}s")
                nc.vector.tensor_scalar(s[:], rng[:], 1.0 / QMAX, EPS,
                                        OP.mult, OP.max)
                inv = cp.tile([P, 1], F32, tag=f"{pref}inv")
                nc.vector.reciprocal(inv[:], s[:])
                bias = cp.tile([P, 1], F32, tag=f"{pref}bias")
                nc.vector.tensor_tensor(bias[:], nmn, inv[:], OP.mult)
                mn = cp.tile([P, 1], F32, tag=f"{pref}mn")
                nc.vector.tensor_scalar(mn[:], nmn, -1.0, None, OP.mult)
                return s, inv, bias, mn

            with tc.high_priority():
                w1s, w1inv, w1bias, w1gmn = qscalars("w1", gw[:, 0:1],
                                                     gw[:, 1:2])
                sx, invsx, biasx, xminv = qscalars("x", gx[:, 0:1],
                                                   gx[:, 1:2])

            # =====================================================
            # qx pass: k = rne((x-xmin)/s) on ACT (int16), dequant into the
            # padded fp16 qxpad. Image 0 in halves (subtile deps) so conv1
            # starts after the top 17 interior rows. w1 quarters interleave
            # so DVE alternates between wl1 prep and the first qx deqs.
            # =====================================================
            def emit_chunk(n0, cnt):
                for a in (0, 1):
                    k = kp.tile([P, cnt, HW], I16, tag=f"k{cnt}",
                                bufs=(8 if cnt == 1 else 4))
                    if n0 == 0:
                        for lo, hi, r0, r1 in ((0, 544, 1, 18),
                                               (544, 1024, 18, 33)):
                            nc.scalar.activation(
                                k[:, 0, lo:hi], x16[:, a, 0, lo:hi], ACTF,
                                bias=biasx[:, 0:1], scale=invsx[:, 0:1])
                            nc.vector.tensor_scalar(
                                qxpad[:, a * NLOC, r0:r1, 1:33],
                                k[:, 0, lo:hi].rearrange(
                                    "p (y x) -> p y x", x=32),
                                sx[:, 0:1], xminv[:, 0:1], OP.mult, OP.add)
                        continue
                    nc.scalar.activation(k[:, :, :], x16[:, a, n0:n0 + cnt, :],
                                         ACTF, bias=biasx[:, 0:1],
                                         scale=invsx[:, 0:1])
                    deng = nc.vector if n0 <= 2 else nc.gpsimd
                    deng.tensor_scalar(
                        qxpad[:, a * NLOC + n0:a * NLOC + n0 + cnt, 1:33, 1:33],
                        k.rearrange("p n (y x) -> p n y x", x=32),
                        sx[:, 0:1], xminv[:, 0:1], OP.mult, OP.add)

            with tc.high_priority():
                w_round_relayout(wl1, w1inv, w1bias, w1s, w1gmn, [(0, 0)],
                                 fast=True)
                emit_chunk(0, 1)
                w_round_relayout(wl1, w1inv, w1bias, w1s, w1gmn, [(0, 1)],
                                 fast=True)
            emit_chunk(1, 1)
            w_round_relayout(wl1, w1inv, w1bias, w1s, w1gmn,
                             [(1, 0), (1, 1)], fast=True)
            emit_chunk(2, 2)
            emit_chunk(4, 2)
            emit_chunk(6, 2)

            # =====================================================
            # conv helper: 18 matmuls per [co_block, half] PSUM tile;
            # -min/max stats straight from PSUM (DVE/Pool), sums via the
            # ACT copy accumulator.
            # =====================================================
            def conv(ns, in_pad_at, wl, outt, sums, mnt, mxt):
                ns = list(ns)
                for n in ns:
                    for cb in (0, 1):
                        for half in (0, 1):
                            ps = pp.tile([P, 512], F32, tag="ps",
                                         bufs=8)
                            i = 0
                            for a in (0, 1):
                                src = in_pad_at(n, a)
                                for ky in (1, 0, 2):
                                    r0, o0 = half * 16 + ky, 0
                                    rows = 16
                                    if ky == 0 and half == 0:
                                        r0, o0, rows = 1, 32, 15
                                    elif ky == 2 and half == 1:
                                        rows = 15
                                    for kx in (1, 0, 2):
                                        c0, x0, cols = kx, 0, 32
                                        if kx == 0:
                                            c0, x0, cols = 1, 1, 31
                                        elif kx == 2:
                                            cols = 31
                                        rhs = src[:, r0:r0 + rows,
                                                  c0:c0 + cols]
                                        pv = ps.rearrange(
                                            "p (y x) -> p y x", x=32)
                                        out = pv[:, o0 // 32:o0 // 32 + rows,
                                                 x0:x0 + cols]
                                        nc.tensor.matmul(
                                            out,
                                            wl[a][:, ky * 3 + kx,
                                                  cb * P:(cb + 1) * P],
                                            rhs, start=(i == 0), stop=(i == 17))
                                        i += 1
                            h = n * 2 + half
                            ob = outt[cb][:, n, half * 512:(half + 1) * 512]
                            with tc.high_priority():
                                # the copy releases the psum bank: it must
                                # outrank streaming ACT work when ready
                                nc.scalar.activation(
                                    ob, ps[:], ACTF,
                                    accum_out=sums[:, cb, h:h + 1])
                            # stats on DVE: from the fp16 copy (cheap 2x,
                            # frees the PSUM bank right after the ACT copy)
                            # except the last image, whose stats read PSUM
                            # directly to shorten the barrier chain.
                            st_src = ps[:] if n == ns[-1] else ob
                            nc.vector.tensor_reduce(
                                mnt[:, cb, h:h + 1], st_src, AX, OP.min,
                                negate=True)
                            nc.vector.tensor_reduce(
                                mxt[:, cb, h:h + 1], ob, AX, OP.max)

            # =====================================================
            # range-norm stats: single AllGather of [P,6] per-channel
            # (sum, -min, max) + local folds -> fused affine params
            # =====================================================
            def bn_params(idx, sums, mnt, mxt, gt, bt):
                pk = cp.tile([P, 6], F32, tag=f"pk{idx}", name=f"pk{idx}")
                for cb in (0, 1):
                    nc.vector.tensor_reduce(pk[:, cb:cb + 1], sums[:, cb, :],
                                            AX, OP.add)
                    nc.vector.tensor_reduce(pk[:, 2 + cb:3 + cb],
                                            mnt[:, cb, :], AX, OP.max)
                    nc.vector.tensor_reduce(pk[:, 4 + cb:5 + cb],
                                            mxt[:, cb, :], AX, OP.max)
                snd_s = dp.tile([P, 2], F32, tag=f"snds{idx}",
                                name=f"snds{idx}")
                snd_m = dp.tile([P, 4], F32, tag=f"sndm{idx}",
                                name=f"sndm{idx}")
                rcv_s = dp.tile([P, 2], F32, tag=f"rcvs{idx}",
                                name=f"rcvs{idx}",
                                addr_space=("Local" if local_cc else "Shared"))
                rcv_m = dp.tile([P, 4], F32, tag=f"rcvm{idx}",
                                name=f"rcvm{idx}",
                                addr_space=("Local" if local_cc else "Shared"))
                nc.sync.dma_start(snd_s[:, :], pk[:, 0:2])
                nc.gpsimd.dma_start(snd_m[:, :], pk[:, 2:6])
                allreduce(OP.add, snd_s, rcv_s)
                allreduce(OP.max, snd_m, rcv_m)
                ssum = cp.tile([P, 2], F32, tag=f"ssum{idx}", name=f"ssum{idx}")
                stm4 = cp.tile([P, 4], F32, tag=f"stm4{idx}", name=f"stm4{idx}")
                nc.gpsimd.dma_start(ssum[:, :], rcv_s[:, :])
                nc.sync.dma_start(stm4[:, :], rcv_m[:, :])

                def t2(tag):
                    return cp.tile([P, 2], F32, tag=f"{tag}{idx}",
                                   name=f"{tag}{idx}")

                # params: a_ = g/(C_N*rng+EPS); b_ = b - a_*mean
                mean = t2("mean")
                nc.vector.tensor_scalar(mean[:], ssum[:, :], 1.0 / N_TOTAL,
                                        None, OP.mult)
                rng = t2("rng")
                nc.vector.tensor_tensor(rng[:], stm4[:, 2:4], stm4[:, 0:2],
                                        OP.add)
                sc = t2("sc")
                nc.gpsimd.tensor_scalar(sc[:], rng[:], C_N, EPS, OP.mult,
                                        OP.add)
                inv = t2("inv")
                nc.vector.reciprocal(inv[:], sc[:])
                a_ = t2("a_")
                nc.vector.tensor_tensor(a_[:], gt[:], inv[:], OP.mult)
                am = t2("am")
                nc.vector.tensor_tensor(am[:], a_[:], mean[:], OP.mult)
                b_ = t2("b_")
                nc.vector.tensor_tensor(b_[:], bt[:], am[:], OP.subtract)
                # per-channel output range: lo = a_*min+b_ = b_ - a_*(-min)
                t1_ = t2("t1_")
                nc.vector.tensor_tensor(t1_[:], a_[:], stm4[:, 0:2], OP.mult)
                lo = t2("lo")
                nc.vector.tensor_tensor(lo[:], b_[:], t1_[:], OP.subtract)
                t2_ = t2("t2_")
                nc.vector.tensor_tensor(t2_[:], a_[:], stm4[:, 2:4], OP.mult)
                hi = t2("hi")
                nc.vector.tensor_tensor(hi[:], t2_[:], b_[:], OP.add)
                lo2 = t2("lo2")
                hi2 = t2("hi2")
                nc.vector.tensor_tensor(lo2[:], lo[:], hi[:], OP.min)
                nc.vector.tensor_tensor(hi2[:], lo[:], hi[:], OP.max)
                pq = t2("pq")
                nc.vector.tensor_reduce(pq[:, 0:1], lo2[:], AX, OP.min,
                                        negate=True)
                nc.vector.tensor_reduce(pq[:, 1:2], hi2[:], AX, OP.max)
                nh = t2("nh")
                nc.gpsimd.partition_all_reduce(nh[:, 0:2], pq[:, 0:2], P, RMAX)

                def y1(tag):
                    return cp.tile([P, 1], F32, tag=f"{tag}{idx}",
                                   name=f"{tag}{idx}")
                ymin = y1("ymin")
                nc.vector.tensor_scalar(ymin[:], nh[:, 0:1], -1.0, None,
                                        OP.mult)
                rngy = y1("rngy")
                nc.vector.tensor_tensor(rngy[:], nh[:, 1:2], nh[:, 0:1],
                                        OP.add)
                sy = y1("sy")
                nc.gpsimd.tensor_scalar(sy[:], rngy[:], 1.0 / QMAX, EPS,
                                        OP.mult, OP.max)
                invsy = y1("invsy")
                nc.vector.reciprocal(invsy[:], sy[:])
                A = t2("A")
                nc.vector.tensor_scalar(A[:], a_[:], invsy[:, 0:1], None,
                                        OP.mult)
                B = t2("B")
                nc.gpsimd.tensor_scalar(B[:], b_[:], ymin[:, 0:1], None,
                                        OP.subtract)
                nc.gpsimd.tensor_scalar(B[:], B[:], invsy[:, 0:1], None,
                                        OP.mult)
                return A, B, sy[:, 0:1], ymin[:, 0:1]

            # ---------- conv1 (w2 prep + gamma/beta loads after the first
            # two images so they run in conv1's engine-idle time) ----------
            sums1 = cp.tile([P, 2, 16], F32, tag="sums1")
            mn1 = cp.tile([P, 2, 16], F32, tag="mn1")
            mx1 = cp.tile([P, 2, 16], F32, tag="mx1")
            nc.vector.memset(sums1[:, :, :], 0.0)
            qx_at = lambda n, a: qxpad[:, a * NLOC + n, :, :]
            conv(range(0, 4), qx_at, wl1, out1, sums1, mn1, mx1)

            # w2 prep + gamma/beta loads sit deep inside conv1's shadow so
            # their DMA dispatches and stat reduces never crowd the
            # startup-critical DVE/Pool queues.
            w_load(w2_d, q2=nc.gpsimd)
            # w2 stats on Pool XYZWC (slow but idle engine) so the startup
            # window's DVE queue stays clean of multi-us blocks
            xg2 = cp.tile([1, 4], F32, tag="xg2")
            wneg = [cp.tile([P, NW], F16, tag=f"wneg_{a}", name=f"wneg_{a}")
                    for a in (0, 1)]
            for a in (0, 1):
                # XYZWC only supports max: negate first to get -min
                nc.gpsimd.tensor_scalar(wneg[a][:, :], wraw[a][:, :], -1.0,
                                        None, OP.mult)
                nc.gpsimd.tensor_reduce(xg2[:, a:a + 1], wneg[a][:, :],
                                        mybir.AxisListType.XYZWC, OP.max)
                nc.gpsimd.tensor_reduce(xg2[:, 2 + a:3 + a], wraw[a][:, :],
                                        mybir.AxisListType.XYZWC, OP.max)
            g2w = cp.tile([1, 2], F32, tag="g2w")
            nc.vector.tensor_reduce(g2w[:, 0:1], xg2[:, 0:2], AX, OP.max)
            nc.vector.tensor_reduce(g2w[:, 1:2], xg2[:, 2:4], AX, OP.max)
            gw2 = cp.tile([P, 2], F32, tag="gw2")
            nc.gpsimd.partition_broadcast(gw2[:, :], g2w[0:1, :])
            w2s, w2inv, w2bias, w2gmn = qscalars("w2", gw2[:, 0:1],
                                                 gw2[:, 1:2])
            w_round_relayout(wl2, w2inv, w2bias, w2s, w2gmn,
                             [(0, 0), (0, 1), (1, 0), (1, 1)])
            gb = {}
            for i, (nm, d) in enumerate(
                    (("g1", g1_d), ("b1", b1_d), ("g2", g2_d), ("b2", b2_d))):
                t = cp.tile([P, 2], F32, tag=f"gb_{nm}", name=f"gb_{nm}")
                nc.sync.dma_start(t[:], d.ap().rearrange("(a p) -> p a", p=P))
                gb[nm] = t

            conv(range(4, NLOC), qx_at, wl1, out1, sums1, mn1, mx1)
            warmup(55)
            A1, B1, sy1, ymin1 = bn_params(1, sums1, mn1, mx1, gb["g1"],
                                           gb["b1"])

            # ---------- z = quant(rangenorm(out1)); conv2 into out1 ----------
            sums2 = cp.tile([P, 2, 16], F32, tag="sums2")
            mn2 = cp.tile([P, 2, 16], F32, tag="mn2")
            mx2 = cp.tile([P, 2, 16], F32, tag="mx2")
            nc.vector.memset(sums2[:, :, :], 0.0)
            zpads = {}

            def get_z(n, a):
                if (n, a) not in zpads:
                    z = zb[(2 * n + a) % 2]
                    k = kp.tile([P, 1, HW], I16, tag="k1", bufs=8)
                    nc.scalar.activation(k[:, 0, 0:544], out1[a][:, n, 0:544],
                                         ACTF, bias=B1[:, a:a + 1],
                                         scale=A1[:, a:a + 1])
                    nc.vector.tensor_scalar(
                        z[:, 1:18, 1:33],
                        k[:, 0, 0:544].rearrange("p (y x) -> p y x", x=32),
                        sy1, ymin1, OP.mult, OP.add)
                    nc.scalar.activation(k[:, 0, 544:1024],
                                         out1[a][:, n, 544:1024], ACTF,
                                         bias=B1[:, a:a + 1],
                                         scale=A1[:, a:a + 1])
                    nc.vector.tensor_scalar(
                        z[:, 18:33, 1:33],
                        k[:, 0, 544:1024].rearrange("p (y x) -> p y x", x=32),
                        sy1, ymin1, OP.mult, OP.add)
                    zpads[(n, a)] = z
                return zpads[(n, a)]

            conv(range(NLOC), get_z, wl2, out1, sums2, mn2, mx2)
            A2, B2, sy2, ymin2 = bn_params(2, sums2, mn2, mx2, gb["g2"],
                                           gb["b2"])

            # ---------- final: out = qx + dequant(rne(A2*conv2+B2)) ----------
            # round (ACT/Pool), dequant in place over the int16 codes
            # (DVE/Pool), residual add (DVE/Pool) into v slots carved from
            # the dead x16 buffer, fp16 out on 2 DMA queues.
            xv = x16.rearrange("p a n s -> p (a n s)")
            voff = 0
            # units (n0, cnt, a) ordered so Pool's rounds (all ready the
            # moment params land) sit at the head of its queue; engine maps
            # balance ACT ~12.5us / Pool ~12us / DVE ~13.5us.
            UNITS = [(0, 1, 0), (0, 1, 1), (1, 1, 0), (1, 1, 1),
                     (2, 2, 0), (2, 2, 1), (4, 2, 0), (4, 2, 1),
                     (6, 1, 0), (6, 1, 1), (7, 1, 0), (7, 1, 1)]
            POOL_ROUND = {5, 10, 11}       # indices into UNITS
            POOL_DEQ = {6, 7}
            ACT_DEQ = set()
            for ui, (n0, cnt, a) in enumerate(UNITS):
                k = kp.tile([P, cnt, HW], I16, tag=f"k{cnt}",
                            bufs=(8 if cnt == 1 else 4))
                if ui in POOL_ROUND:
                    nc.gpsimd.tensor_scalar(k[:, :, :],
                                            out1[a][:, n0:n0 + cnt, :],
                                            A2[:, a:a + 1], B2[:, a:a + 1],
                                            OP.mult, OP.add)
                else:
                    nc.scalar.activation(k[:, :, :],
                                         out1[a][:, n0:n0 + cnt, :],
                                         ACTF, bias=B2[:, a:a + 1],
                                         scale=A2[:, a:a + 1])
                u = k.bitcast(F16)
                if ui in ACT_DEQ:
                    nc.scalar.activation(u[:, :, :], k[:, :, :], ACTF,
                                         bias=ymin2, scale=sy2)
                else:
                    ueng = nc.gpsimd if ui in POOL_DEQ else nc.vector
                    ueng.tensor_scalar(u[:, :, :], k[:, :, :],
                                       sy2, ymin2, OP.mult, OP.add)
                if voff + cnt * HW > 16384:
                    voff = 0
                v = xv[:, voff:voff + cnt * HW]
                voff += cnt * HW
                nc.vector.tensor_tensor(
                    v.rearrange("p (n y x) -> p n y x", y=32, x=32),
                    u.rearrange("p n (y x) -> p n y x", x=32),
                    qxpad[:, a * NLOC + n0:a * NLOC + n0 + cnt, 1:33, 1:33],
                    OP.add)
                nc.sync.dma_start(
                    out_d.ap()[n0:n0 + cnt, a * P:(a + 1) * P, :]
                    .rearrange("n c h -> c n h"),
                    v.rearrange("p (n h) -> p n h", n=cnt))

    nc.compile()
    return nc


def kernel(**inputs):
    global _cached_nc
    from concourse import bass_utils

    x = np.ascontiguousarray(
        np.asarray(inputs["x"], dtype=np.float32).reshape(64, C, HW)
        .astype(np.float16))
    w1 = np.asarray(inputs["w1"], dtype=np.float32).reshape(C, C, 9)
    w2 = np.asarray(inputs["w2"], dtype=np.float32).reshape(C, C, 9)
    # [co, ci, q] -> [ci, q, co], fp16
    w1t = np.ascontiguousarray(w1.transpose(1, 2, 0).astype(np.float16))
    w2t = np.ascontiguousarray(w2.transpose(1, 2, 0).astype(np.float16))
    g1 = np.ascontiguousarray(np.asarray(inputs["gamma1"], dtype=np.float32))
    b1 = np.ascontiguousarray(np.asarray(inputs["beta1"], dtype=np.float32))
    g2 = np.ascontiguousarray(np.asarray(inputs["gamma2"], dtype=np.float32))
    b2 = np.ascontiguousarray(np.asarray(inputs["beta2"], dtype=np.float32))

    if _cached_nc is None:
        _cached_nc = _build()
    nc = _cached_nc

    in_maps = []
    for cid in range(N_CORES):
        in_maps.append({
            "x": np.ascontiguousarray(x[cid * NLOC:(cid + 1) * NLOC]),
            "w1t": w1t, "w2t": w2t,
            "gamma1": g1, "beta1": b1, "gamma2": g2, "beta2": b2,
        })
    res = bass_utils.run_bass_kernel_spmd(
        nc, in_maps, core_ids=list(range(N_CORES)))
    out = np.concatenate(
        [res.results[cid]["out"].reshape(NLOC, C, 32, 32).astype(np.float32)
         for cid in range(N_CORES)],
        axis=0)
    kernel.last_results = res
    return out


# revision 51
# speedup vs baseline: 1.1417x; 1.0110x over previous
"""Trainium2 Bass kernel for a quantized BasicBlock (QConv3x3 -> RangeNorm ->
QConv3x3 -> RangeNorm -> quantized residual add).

Sharding: data-parallel over batch (8 images per core across 8 cores);
weights replicated; per-tensor quantization min/max and per-channel
range-norm stats combined across cores with small collectives.

Per core (v2):
  - x uploaded fp16 (halves the input DMA) straight into the resident x16
    buffer in 16 half-image chunks on 2 queues. Per-chunk -min on DVE
    (flat reduces); max via a Pool TT-tree over chunks 0..11 (scratch in
    the not-yet-written out1 buffers) + flat DVE reduces for chunks 12..15.
    One packed partition_all_reduce + one AllReduce(max) of (-min, max).
  - w1/w2 uploaded fp16 in [ci, 9, co] layout; block min/max on DVE/Pool,
    round to int16 codes on Pool (RNE via dtype convert), dequant-relayout
    to fp16 on DVE (packed 2x), split per co-half so conv1 starts early.
  - qx pass: k = rne((x-xmin)/s) on ACT (fp16 -> int16), dequant to the
    zero-halo-free padded fp16 qxpad on DVE/Pool. Image 0 in halves so
    conv1's first tile starts after 17 interior rows.
  - conv3x3 = 18 accumulating PE matmuls per [co_block, half-image] PSUM
    tile; image-edge taps skip halo-only rows/cols (halo never read, so
    no halo memsets). Per-tile -min/max read directly from PSUM on
    DVE/Pool; sums via the ACT psum->SBUF copy accumulator.
  - RangeNorm stats: one AllGather of [128,6] per-channel (sum,-min,max)
    followed by local folds (sum-add / max) and a short two-engine param
    chain; bn output quantizer scale derived analytically.
  - z = quant(rangenorm(out1)) built in halves as int16 codes + fp16
    dequant; conv2 reuses out1.
  - final: out = qx + dequant(rne(A2*conv2 + B2)); round/dequant/add
    rotated across ACT/DVE/Pool, int16 codes dequantized in place, fp16
    result DMA'd out on 2 queues (fp16 download, upcast on host).
"""

import os
import numpy as np

N_CORES = 8
NLOC = 8            # images per core
C = 256
P = 128
HW = 1024           # 32*32
PAD = 34            # 32+2
EPS = 1e-8
QMAX = 255.0
N_TOTAL = 64 * 32 * 32          # range-norm n (global batch)
C_N = float(1.0 / np.sqrt(2.0 * np.log(N_TOTAL)))
NW = C * 9          # w block free size (per ci block)

_cached_nc = None


def _build(sim_single=False, no_collectives=False):
    """sim_single=True builds a 1-core variant with collectives replaced by
    a stand-in DMA — numerically wrong across cores but structurally
    identical, for TimelineSim cost-model analysis. no_collectives=True keeps
    8 cores but swaps collectives for local DMAs (timing A/B only)."""
    import concourse.bass as bass
    import concourse.mybir as mybir
    from concourse import bacc, tile
    import concourse.bass_isa as bass_isa

    dt = mybir.dt
    F32, F16, I16 = dt.float32, dt.float16, dt.int16
    AX = mybir.AxisListType.X
    AXY = mybir.AxisListType.XY
    OP = mybir.AluOpType
    ACTF = mybir.ActivationFunctionType.Identity
    RMAX = bass_isa.ReduceOp.max

    nc = bacc.Bacc("TRN2", target_bir_lowering=False, debug=False,
                   num_devices=(1 if sim_single else N_CORES))

    local_cc = sim_single or no_collectives

    def _flat(ap):
        names = "abcde"[:len(ap.shape)]
        if len(names) == 1:
            return ap
        spec = " ".join(names)
        return ap.rearrange(f"{spec} -> ({spec})")

    def allreduce(op, snd, rcv):
        if local_cc:
            nc.sync.dma_start(_flat(rcv)[None, :], _flat(snd)[None, :])
        else:
            nc.gpsimd.collective_compute(
                "AllReduce", op,
                replica_groups=[list(range(N_CORES))],
                ins=[snd.opt()], outs=[rcv.opt()])

    def allgather(snd, rcv, nelem):
        if local_cc:
            nc.sync.dma_start(_flat(rcv)[0:nelem][None, :], _flat(snd)[None, :])
        else:
            nc.gpsimd.collective_compute(
                "AllGather", mybir.AluOpType.bypass,
                replica_groups=[list(range(N_CORES))],
                ins=[snd.opt()], outs=[rcv.opt()])

    x_d = nc.dram_tensor("x", [NLOC, C, HW], F16, kind="ExternalInput")
    w1_d = nc.dram_tensor("w1t", [C, 9, C], F16, kind="ExternalInput")
    w2_d = nc.dram_tensor("w2t", [C, 9, C], F16, kind="ExternalInput")
    g1_d = nc.dram_tensor("gamma1", [C], F32, kind="ExternalInput")
    b1_d = nc.dram_tensor("beta1", [C], F32, kind="ExternalInput")
    g2_d = nc.dram_tensor("gamma2", [C], F32, kind="ExternalInput")
    b2_d = nc.dram_tensor("beta2", [C], F32, kind="ExternalInput")
    out_d = nc.dram_tensor("out", [NLOC, C, HW], F16, kind="ExternalOutput")

    with tile.TileContext(nc) as tc:
        with tc.tile_pool(name="consts", bufs=1) as cp, \
             tc.tile_pool(name="dram", bufs=1, space="DRAM") as dp, \
             tc.tile_pool(name="psum", bufs=8, space="PSUM") as pp, \
             tc.tile_pool(name="ktmp", bufs=3) as kp:

            # ---------- persistent tiles ----------
            qxpad = cp.tile([P, 2 * NLOC, PAD, PAD], F16, tag="qxpad")
            x16 = cp.tile([P, 2, NLOC, HW], F16, tag="x16", name="x16")
            out1 = [cp.tile([P, NLOC, HW], F16, tag=f"out1_{a}",
                            name=f"out1_{a}") for a in (0, 1)]
            wl1 = [cp.tile([P, 9, C], F16, tag=f"wl1_{a}", name=f"wl1_{a}")
                   for a in (0, 1)]
            wl2 = [cp.tile([P, 9, C], F16, tag=f"wl2_{a}", name=f"wl2_{a}")
                   for a in (0, 1)]
            zb = [cp.tile([P, PAD, PAD], F16, tag=f"zb_{i}", name=f"zb_{i}")
                  for i in range(2)]
            wraw = [cp.tile([P, NW], F16, tag=f"wraw_{a}", name=f"wraw_{a}")
                    for a in (0, 1)]
            kw = [cp.tile([P, 9, C], I16, tag=f"kw_{a}", name=f"kw_{a}")
                  for a in (0, 1)]

            # PE warmup: dummy matmuls keep the tensor engine's p-state
            # ramped through the startup and inter-conv stat barriers
            # (PE is otherwise idle there, so they cost nothing).
            wtile = cp.tile([P, 512], F16, tag="wtile")
            nc.vector.memset(wtile[:, :], 0.0)

            def warmup(n):
                # rotate through the conv psum banks (idle while PE idles)
                # so consecutive warmups overlap their WAW semaphores
                for _ in range(n):
                    pw = pp.tile([P, 512], F32, tag="ps", bufs=8)
                    nc.tensor.matmul(pw[:, :], wtile[:, 0:P], wtile[:, :],
                                     start=True, stop=True)

            warmup(126)

            # =====================================================
            # x stream: 16 half-image fp16 chunks straight into x16.
            # min: DVE TT-tree (fp16 2x) with a fused ttr top. max: Pool
            # XYZWC scalar reduces for chunks 0..11 + a DVE TT-tree for
            # chunks 12..15. w1 block min/max (DVE ttr trees) ride the
            # same single AllReduce(max) of [-xmin, xmax, -w1min, w1max].
            # Tree scratch lands in the idle out1 buffers.
            # =====================================================
            ps4 = cp.tile([P, 4], F32, tag="ps4")    # -min, max, -w1n, w1x
            xg = cp.tile([1, 12], F32, tag="xg")     # Pool chunk maxima 0..11
            o1v = [out1[i].rearrange("p n s -> p (n s)") for i in (0, 1)]

            def mslot(i):
                """[P,1024] tree scratch: slots 0-7 in out1[0], 8+ in
                out1[1] (both unwritten until conv1's psum copies)."""
                return o1v[i // 8][:, (i % 8) * HW:((i % 8) + 1) * HW]

            def xch(c):
                return x16[:, c % 2, c // 2, :]

            for j in range(NLOC):
                for a in (0, 1):
                    c = 2 * j + a
                    eng = nc.sync if a == 0 else nc.scalar
                    eng.dma_start(x16[:, a, j, :], x_d.ap()[j, a * P:(a + 1) * P, :])
                    if c <= 11:
                        nc.gpsimd.tensor_reduce(
                            xg[:, c:c + 1], xch(c),
                            mybir.AxisListType.XYZWC, OP.max)
                # min tree leaf for the image pair
                nc.vector.tensor_tensor(mslot(j), xch(2 * j), xch(2 * j + 1),
                                        OP.min)
                if j % 2 == 1:
                    nc.vector.tensor_tensor(mslot(8 + j // 2), mslot(j - 1),
                                            mslot(j), OP.min)
                if j >= 6:
                    # max-side leaves for chunks 12..15 (Pool covers 0..11)
                    nc.vector.tensor_tensor(mslot(14 + (j - 6)), xch(2 * j),
                                            xch(2 * j + 1), OP.max)
            # min tree top over the 4 uppers -> -min
            nc.vector.tensor_tensor(mslot(12), mslot(8), mslot(9), OP.min)
            nc.vector.tensor_tensor(mslot(13), mslot(10), mslot(11), OP.min)
            nc.vector.tensor_tensor(mslot(0), mslot(12), mslot(13), OP.min)
            nc.vector.tensor_reduce(ps4[:, 0:1], mslot(0), AX, OP.min,
                                    negate=True)
            # max tree top over chunks 12..15
            nc.vector.tensor_tensor(mslot(5), mslot(14), mslot(15), OP.max)
            nc.vector.tensor_reduce(ps4[:, 1:2], mslot(5), AX, OP.max)

            # x collective fires as soon as local x stats land; the w1
            # pipeline overlaps its round-trip latency.
            snd_x = dp.tile([2], F32, tag="snd_x")
            rcv_x = dp.tile([2], F32, tag="rcv_x",
                            addr_space=("Local" if local_cc else "Shared"))
            gxp = cp.tile([P, 2], F32, tag="gxp")
            xgf = cp.tile([1, 1], F32, tag="xgf")
            nc.vector.tensor_reduce(xgf[:, :], xg[:, :], AX, OP.max)
            nc.gpsimd.partition_all_reduce(gxp[:, 0:2], ps4[:, 0:2], P, RMAX)
            nc.vector.tensor_tensor(gxp[0:1, 1:2], gxp[0:1, 1:2],
                                    xgf[0:1, :], OP.max)
            nc.sync.dma_start(snd_x[None, :], gxp[0:1, 0:2])
            allreduce(OP.max, snd_x, rcv_x)
            gx = cp.tile([P, 2], F32, tag="gx")
            nc.scalar.dma_start(gx[:, :], rcv_x[None, :].broadcast_to([P, 2]))

            # =====================================================
            # w1: fp16 [ci, 9, co] in 4 pieces per ci block on both queues
            # (behind x on the bus); block stats DVE/Pool; round to int16 on
            # Pool; dequant-relayout on DVE split per co-half so conv1's
            # first tiles aren't gated on the whole weight pipeline.
            # =====================================================
            def w_load(w_dram, q2=None):
                # q2: engine for the odd DMA queue. w1 streams while ACT is
                # idle (scalar queue); w2 must stay off the ACT SEQ so its
                # dispatch never blocks the latency-critical k rounds.
                q2 = q2 or nc.scalar
                for a in (0, 1):
                    src = w_dram.ap()[a * P:(a + 1) * P, :, :].rearrange(
                        "p q c -> p (q c)")
                    step = NW // 4
                    for piece in range(4):
                        lo = piece * step
                        eng = nc.sync if (piece + a) % 2 == 0 else q2
                        eng.dma_start(wraw[a][:, lo:lo + step],
                                      src[:, lo:lo + step])

            def w_stats(idx):
                wst = cp.tile([P, 4], F32, tag=f"wst{idx}", name=f"wst{idx}")
                # cols: [-min a0, -min a1, max a0, max a1]
                nc.vector.tensor_reduce(wst[:, 0:1], wraw[0][:, :], AX,
                                        OP.min, negate=True)
                nc.vector.tensor_reduce(wst[:, 2:3], wraw[0][:, :], AX, OP.max)
                nc.vector.tensor_reduce(wst[:, 1:2], wraw[1][:, :], AX,
                                        OP.min, negate=True)
                nc.vector.tensor_reduce(wst[:, 3:4], wraw[1][:, :], AX, OP.max)
                return wst

            def w_scalars(idx, wst):
                def s1(tag):
                    return cp.tile([P, 1], F32, tag=f"{tag}{idx}",
                                   name=f"{tag}{idx}")
                pnx = cp.tile([P, 2], F32, tag=f"wpnx{idx}", name=f"wpnx{idx}")
                nc.vector.tensor_reduce(pnx[:, 0:1], wst[:, 0:2], AX, OP.max)
                nc.vector.tensor_reduce(pnx[:, 1:2], wst[:, 2:4], AX, OP.max)
                gw = cp.tile([P, 2], F32, tag=f"wgw{idx}", name=f"wgw{idx}")
                nc.gpsimd.partition_all_reduce(gw[:, 0:2], pnx[:, 0:2], P, RMAX)
                rng = s1("wrng")
                nc.vector.tensor_tensor(rng[:], gw[:, 1:2], gw[:, 0:1], OP.add)
                s = s1("ws_")
                nc.vector.tensor_scalar(s[:], rng[:], 1.0 / QMAX, EPS,
                                        OP.mult, OP.max)
                inv = s1("winv")
                nc.vector.reciprocal(inv[:], s[:])
                bias = s1("wbias")
                nc.vector.tensor_tensor(bias[:], gw[:, 0:1], inv[:], OP.mult)
                gmn = s1("wgmn")
                nc.vector.tensor_scalar(gmn[:], gw[:, 0:1], -1.0, None, OP.mult)
                return s, inv, bias, gmn

            def w_round_relayout(wl, inv, bias, s, gmn, quarters,
                                 fast=False):
                # round to int16 codes + dequant-relayout to fp16 per
                # (co-half, ci-block). fast: cb0 rounds on DVE (2x int16)
                # for the conv1-critical quarters; otherwise Pool.
                wrv = [wraw[a].rearrange("p (q c) -> p q c", c=C)
                       for a in (0, 1)]
                for cb, a in quarters:
                    cs = slice(cb * P, (cb + 1) * P)
                    reng = nc.vector if (fast and cb == 0) else nc.gpsimd
                    reng.tensor_scalar(
                        kw[a][:, :, cs], wrv[a][:, :, cs],
                        inv[:, 0:1], bias[:, 0:1], OP.mult, OP.add)
                    nc.vector.tensor_scalar(
                        wl[a][:, :, cs], kw[a][:, :, cs],
                        s[:, 0:1], gmn[:, 0:1], OP.mult, OP.add)

            # w1 stats: weights replicated, so local stats are already
            # global — no collective, just a partition all-reduce. These
            # DVE trees overlap the x collective's round-trip latency.
            w_load(w1_d)
            wt = [cp.tile([P, NW // 2], F16, tag=f"wt{i}", name=f"wt{i}")
                  for i in range(3)]
            for side, (op, col) in enumerate(((OP.min, 2), (OP.max, 3))):
                nc.vector.tensor_tensor(wt[0][:, :], wraw[0][:, 0:NW // 2],
                                        wraw[0][:, NW // 2:NW], op)
                nc.vector.tensor_tensor(wt[1][:, :], wraw[1][:, 0:NW // 2],
                                        wraw[1][:, NW // 2:NW], op)
                nc.vector.tensor_tensor(wt[2][:, :], wt[0][:, :],
                                        wt[1][:, :], op)
                nc.vector.tensor_reduce(ps4[:, col:col + 1], wt[2][:, :], AX,
                                        op, negate=(op == OP.min))
            gw = cp.tile([P, 2], F32, tag="gw")
            nc.gpsimd.partition_all_reduce(gw[:, 0:2], ps4[:, 2:4], P, RMAX)

            # ---------- w1 + x quant scalars ----------
            def qscalars(pref, nmn, mx):
                rng = cp.tile([P, 1], F32, tag=f"{pref}rng")
                nc.vector.tensor_tensor(rng[:], mx, nmn, OP.add)
                s = cp.tile([P, 1], F32, tag=f"{pref}s")
                nc.vector.tensor_scalar(s[:], rng[:], 1.0 / QMAX, EPS,
                                        OP.mult, OP.max)
                inv = cp.tile([P, 1], F32, tag=f"{pref}inv")
                nc.vector.reciprocal(inv[:], s[:])
                bias = cp.tile([P, 1], F32, tag=f"{pref}bias")
                nc.vector.tensor_tensor(bias[:], nmn, inv[:], OP.mult)
                mn = cp.tile([P, 1], F32, tag=f"{pref}mn")
                nc.vector.tensor_scalar(mn[:], nmn, -1.0, None, OP.mult)
                return s, inv, bias, mn

            with tc.high_priority():
                w1s, w1inv, w1bias, w1gmn = qscalars("w1", gw[:, 0:1],
                                                     gw[:, 1:2])
                sx, invsx, biasx, xminv = qscalars("x", gx[:, 0:1],
                                                   gx[:, 1:2])

            # =====================================================
            # qx pass: k = rne((x-xmin)/s) on ACT (int16), dequant into the
            # padded fp16 qxpad. Image 0 in halves (subtile deps) so conv1
            # starts after the top 17 interior rows. w1 quarters interleave
            # so DVE alternates between wl1 prep and the first qx deqs.
            # =====================================================
            def emit_chunk(n0, cnt):
                for a in (0, 1):
                    k = kp.tile([P, cnt, HW], I16, tag=f"k{cnt}",
                                bufs=(8 if cnt == 1 else 4))
                    if n0 == 0:
                        for lo, hi, r0, r1 in ((0, 544, 1, 18),
                                               (544, 1024, 18, 33)):
                            nc.scalar.activation(
                                k[:, 0, lo:hi], x16[:, a, 0, lo:hi], ACTF,
                                bias=biasx[:, 0:1], scale=invsx[:, 0:1])
                            nc.vector.tensor_scalar(
                                qxpad[:, a * NLOC, r0:r1, 1:33],
                                k[:, 0, lo:hi].rearrange(
                                    "p (y x) -> p y x", x=32),
                                sx[:, 0:1], xminv[:, 0:1], OP.mult, OP.add)
                        continue
                    nc.scalar.activation(k[:, :, :], x16[:, a, n0:n0 + cnt, :],
                                         ACTF, bias=biasx[:, 0:1],
                                         scale=invsx[:, 0:1])
                    deng = nc.vector if n0 <= 2 else nc.gpsimd
                    deng.tensor_scalar(
                        qxpad[:, a * NLOC + n0:a * NLOC + n0 + cnt, 1:33, 1:33],
                        k.rearrange("p n (y x) -> p n y x", x=32),
                        sx[:, 0:1], xminv[:, 0:1], OP.mult, OP.add)

            with tc.high_priority():
                w_round_relayout(wl1, w1inv, w1bias, w1s, w1gmn, [(0, 0)],
                                 fast=True)
                emit_chunk(0, 1)
                w_round_relayout(wl1, w1inv, w1bias, w1s, w1gmn, [(0, 1)],
                                 fast=True)
            emit_chunk(1, 1)
            w_round_relayout(wl1, w1inv, w1bias, w1s, w1gmn,
                             [(1, 0), (1, 1)], fast=True)
            emit_chunk(2, 2)
            emit_chunk(4, 2)
            emit_chunk(6, 2)

            # =====================================================
            # conv helper: 18 matmuls per [co_block, half] PSUM tile;
            # -min/max stats straight from PSUM (DVE/Pool), sums via the
            # ACT copy accumulator.
            # =====================================================
            def conv(ns, in_pad_at, wl, outt, sums, mnt, mxt):
                ns = list(ns)
                for n in ns:
                    for cb in (0, 1):
                        for half in (0, 1):
                            ps = pp.tile([P, 512], F32, tag="ps",
                                         bufs=8)
                            i = 0
                            for a in (0, 1):
                                src = in_pad_at(n, a)
                                for ky in (1, 0, 2):
                                    r0, o0 = half * 16 + ky, 0
                                    rows = 16
                                    if ky == 0 and half == 0:
                                        r0, o0, rows = 1, 32, 15
                                    elif ky == 2 and half == 1:
                                        rows = 15
                                    for kx in (1, 0, 2):
                                        c0, x0, cols = kx, 0, 32
                                        if kx == 0:
                                            c0, x0, cols = 1, 1, 31
                                        elif kx == 2:
                                            cols = 31
                                        rhs = src[:, r0:r0 + rows,
                                                  c0:c0 + cols]
                                        pv = ps.rearrange(
                                            "p (y x) -> p y x", x=32)
                                        out = pv[:, o0 // 32:o0 // 32 + rows,
                                                 x0:x0 + cols]
                                        nc.tensor.matmul(
                                            out,
                                            wl[a][:, ky * 3 + kx,
                                                  cb * P:(cb + 1) * P],
                                            rhs, start=(i == 0), stop=(i == 17))
                                        i += 1
                            h = n * 2 + half
                            ob = outt[cb][:, n, half * 512:(half + 1) * 512]
                            with tc.high_priority():
                                # the copy releases the psum bank: it must
                                # outrank streaming ACT work when ready
                                nc.scalar.activation(
                                    ob, ps[:], ACTF,
                                    accum_out=sums[:, cb, h:h + 1])
                            # stats on DVE: from the fp16 copy (cheap 2x,
                            # frees the PSUM bank right after the ACT copy)
                            # except the last image, whose stats read PSUM
                            # directly to shorten the barrier chain.
                            st_src = ps[:] if n == ns[-1] else ob
                            nc.vector.tensor_reduce(
                                mnt[:, cb, h:h + 1], st_src, AX, OP.min,
                                negate=True)
                            nc.vector.tensor_reduce(
                                mxt[:, cb, h:h + 1], ob, AX, OP.max)

            # =====================================================
            # range-norm stats: single AllGather of [P,6] per-channel
            # (sum, -min, max) + local folds -> fused affine params
            # =====================================================
            def bn_params(idx, sums, mnt, mxt, gt, bt):
                with tc.high_priority():
                    return _bn_params(idx, sums, mnt, mxt, gt, bt)

            def _bn_params(idx, sums, mnt, mxt, gt, bt):
                pk = cp.tile([P, 6], F32, tag=f"pk{idx}", name=f"pk{idx}")
                for cb in (0, 1):
                    nc.vector.tensor_reduce(pk[:, cb:cb + 1], sums[:, cb, :],
                                            AX, OP.add)
                    nc.vector.tensor_reduce(pk[:, 2 + cb:3 + cb],
                                            mnt[:, cb, :], AX, OP.max)
                    nc.vector.tensor_reduce(pk[:, 4 + cb:5 + cb],
                                            mxt[:, cb, :], AX, OP.max)
                snd_s = dp.tile([P, 2], F32, tag=f"snds{idx}",
                                name=f"snds{idx}")
                snd_m = dp.tile([P, 4], F32, tag=f"sndm{idx}",
                                name=f"sndm{idx}")
                rcv_s = dp.tile([P, 2], F32, tag=f"rcvs{idx}",
                                name=f"rcvs{idx}",
                                addr_space=("Local" if local_cc else "Shared"))
                rcv_m = dp.tile([P, 4], F32, tag=f"rcvm{idx}",
                                name=f"rcvm{idx}",
                                addr_space=("Local" if local_cc else "Shared"))
                # the max-side send gates the whole param chain: issue it
                # first on the idle sync queue (Pool's DMA dispatch costs
                # ~1us of its sequencer)
                nc.sync.dma_start(snd_m[:, :], pk[:, 2:6])
                nc.gpsimd.dma_start(snd_s[:, :], pk[:, 0:2])
                allreduce(OP.max, snd_m, rcv_m)
                allreduce(OP.add, snd_s, rcv_s)
                ssum = cp.tile([P, 2], F32, tag=f"ssum{idx}", name=f"ssum{idx}")
                stm4 = cp.tile([P, 4], F32, tag=f"stm4{idx}", name=f"stm4{idx}")
                nc.sync.dma_start(stm4[:, :], rcv_m[:, :])
                nc.gpsimd.dma_start(ssum[:, :], rcv_s[:, :])

                def t2(tag):
                    return cp.tile([P, 2], F32, tag=f"{tag}{idx}",
                                   name=f"{tag}{idx}")

                # params: a_ = g/(C_N*rng+EPS); b_ = b - a_*mean
                mean = t2("mean")
                nc.vector.tensor_scalar(mean[:], ssum[:, :], 1.0 / N_TOTAL,
                                        None, OP.mult)
                rng = t2("rng")
                nc.vector.tensor_tensor(rng[:], stm4[:, 2:4], stm4[:, 0:2],
                                        OP.add)
                sc = t2("sc")
                nc.gpsimd.tensor_scalar(sc[:], rng[:], C_N, EPS, OP.mult,
                                        OP.add)
                inv = t2("inv")
                nc.vector.reciprocal(inv[:], sc[:])
                a_ = t2("a_")
                nc.vector.tensor_tensor(a_[:], gt[:], inv[:], OP.mult)
                am = t2("am")
                nc.vector.tensor_tensor(am[:], a_[:], mean[:], OP.mult)
                b_ = t2("b_")
                nc.vector.tensor_tensor(b_[:], bt[:], am[:], OP.subtract)
                # per-channel output range: lo = a_*min+b_ = b_ - a_*(-min)
                t1_ = t2("t1_")
                nc.vector.tensor_tensor(t1_[:], a_[:], stm4[:, 0:2], OP.mult)
                lo = t2("lo")
                nc.vector.tensor_tensor(lo[:], b_[:], t1_[:], OP.subtract)
                t2_ = t2("t2_")
                nc.vector.tensor_tensor(t2_[:], a_[:], stm4[:, 2:4], OP.mult)
                hi = t2("hi")
                nc.vector.tensor_tensor(hi[:], t2_[:], b_[:], OP.add)
                lo2 = t2("lo2")
                hi2 = t2("hi2")
                nc.vector.tensor_tensor(lo2[:], lo[:], hi[:], OP.min)
                nc.vector.tensor_tensor(hi2[:], lo[:], hi[:], OP.max)
                pq = t2("pq")
                nc.vector.tensor_reduce(pq[:, 0:1], lo2[:], AX, OP.min,
                                        negate=True)
                nc.vector.tensor_reduce(pq[:, 1:2], hi2[:], AX, OP.max)
                nh = t2("nh")
                nc.gpsimd.partition_all_reduce(nh[:, 0:2], pq[:, 0:2], P, RMAX)

                def y1(tag):
                    return cp.tile([P, 1], F32, tag=f"{tag}{idx}",
                                   name=f"{tag}{idx}")
                ymin = y1("ymin")
                nc.vector.tensor_scalar(ymin[:], nh[:, 0:1], -1.0, None,
                                        OP.mult)
                rngy = y1("rngy")
                nc.vector.tensor_tensor(rngy[:], nh[:, 1:2], nh[:, 0:1],
                                        OP.add)
                sy = y1("sy")
                nc.gpsimd.tensor_scalar(sy[:], rngy[:], 1.0 / QMAX, EPS,
                                        OP.mult, OP.max)
                invsy = y1("invsy")
                nc.vector.reciprocal(invsy[:], sy[:])
                A = t2("A")
                nc.vector.tensor_scalar(A[:], a_[:], invsy[:, 0:1], None,
                                        OP.mult)
                B = t2("B")
                nc.gpsimd.tensor_scalar(B[:], b_[:], ymin[:, 0:1], None,
                                        OP.subtract)
                nc.gpsimd.tensor_scalar(B[:], B[:], invsy[:, 0:1], None,
                                        OP.mult)
                return A, B, sy[:, 0:1], ymin[:, 0:1]

            # ---------- conv1 (w2 prep + gamma/beta loads after the first
            # two images so they run in conv1's engine-idle time) ----------
            sums1 = cp.tile([P, 2, 16], F32, tag="sums1")
            mn1 = cp.tile([P, 2, 16], F32, tag="mn1")
            mx1 = cp.tile([P, 2, 16], F32, tag="mx1")
            nc.vector.memset(sums1[:, :, :], 0.0)
            qx_at = lambda n, a: qxpad[:, a * NLOC + n, :, :]
            conv(range(0, 4), qx_at, wl1, out1, sums1, mn1, mx1)

            # w2 prep + gamma/beta loads sit deep inside conv1's shadow so
            # their DMA dispatches and stat reduces never crowd the
            # startup-critical DVE/Pool queues.
            w_load(w2_d, q2=nc.gpsimd)
            # w2 stats on Pool XYZWC (slow but idle engine) so the startup
            # window's DVE queue stays clean of multi-us blocks
            xg2 = cp.tile([1, 4], F32, tag="xg2")
            wneg = [cp.tile([P, NW], F16, tag=f"wneg_{a}", name=f"wneg_{a}")
                    for a in (0, 1)]
            for a in (0, 1):
                # XYZWC only supports max: negate first to get -min
                nc.gpsimd.tensor_scalar(wneg[a][:, :], wraw[a][:, :], -1.0,
                                        None, OP.mult)
                nc.gpsimd.tensor_reduce(xg2[:, a:a + 1], wneg[a][:, :],
                                        mybir.AxisListType.XYZWC, OP.max)
                nc.gpsimd.tensor_reduce(xg2[:, 2 + a:3 + a], wraw[a][:, :],
                                        mybir.AxisListType.XYZWC, OP.max)
            g2w = cp.tile([1, 2], F32, tag="g2w")
            nc.vector.tensor_reduce(g2w[:, 0:1], xg2[:, 0:2], AX, OP.max)
            nc.vector.tensor_reduce(g2w[:, 1:2], xg2[:, 2:4], AX, OP.max)
            gw2 = cp.tile([P, 2], F32, tag="gw2")
            nc.gpsimd.partition_broadcast(gw2[:, :], g2w[0:1, :])
            w2s, w2inv, w2bias, w2gmn = qscalars("w2", gw2[:, 0:1],
                                                 gw2[:, 1:2])
            w_round_relayout(wl2, w2inv, w2bias, w2s, w2gmn,
                             [(0, 0), (0, 1), (1, 0), (1, 1)])
            gb = {}
            for i, (nm, d) in enumerate(
                    (("g1", g1_d), ("b1", b1_d), ("g2", g2_d), ("b2", b2_d))):
                t = cp.tile([P, 2], F32, tag=f"gb_{nm}", name=f"gb_{nm}")
                nc.sync.dma_start(t[:], d.ap().rearrange("(a p) -> p a", p=P))
                gb[nm] = t

            conv(range(4, NLOC), qx_at, wl1, out1, sums1, mn1, mx1)
            warmup(51)
            A1, B1, sy1, ymin1 = bn_params(1, sums1, mn1, mx1, gb["g1"],
                                           gb["b1"])

            # ---------- z = quant(rangenorm(out1)); conv2 into out1 ----------
            sums2 = cp.tile([P, 2, 16], F32, tag="sums2")
            mn2 = cp.tile([P, 2, 16], F32, tag="mn2")
            mx2 = cp.tile([P, 2, 16], F32, tag="mx2")
            nc.vector.memset(sums2[:, :, :], 0.0)
            zpads = {}

            def get_z(n, a):
                if (n, a) not in zpads:
                    z = zb[(2 * n + a) % 2]
                    k = kp.tile([P, 1, HW], I16, tag="k1", bufs=8)
                    nc.scalar.activation(k[:, 0, 0:544], out1[a][:, n, 0:544],
                                         ACTF, bias=B1[:, a:a + 1],
                                         scale=A1[:, a:a + 1])
                    nc.vector.tensor_scalar(
                        z[:, 1:18, 1:33],
                        k[:, 0, 0:544].rearrange("p (y x) -> p y x", x=32),
                        sy1, ymin1, OP.mult, OP.add)
                    nc.scalar.activation(k[:, 0, 544:1024],
                                         out1[a][:, n, 544:1024], ACTF,
                                         bias=B1[:, a:a + 1],
                                         scale=A1[:, a:a + 1])
                    nc.vector.tensor_scalar(
                        z[:, 18:33, 1:33],
                        k[:, 0, 544:1024].rearrange("p (y x) -> p y x", x=32),
                        sy1, ymin1, OP.mult, OP.add)
                    zpads[(n, a)] = z
                return zpads[(n, a)]

            conv(range(NLOC), get_z, wl2, out1, sums2, mn2, mx2)
            A2, B2, sy2, ymin2 = bn_params(2, sums2, mn2, mx2, gb["g2"],
                                           gb["b2"])

            # ---------- final: out = qx + dequant(rne(A2*conv2+B2)) ----------
            # round (ACT/Pool), dequant in place over the int16 codes
            # (DVE/Pool), residual add (DVE/Pool) into v slots carved from
            # the dead x16 buffer, fp16 out on 2 DMA queues.
            xv = x16.rearrange("p a n s -> p (a n s)")
            voff = 0
            # units (n0, cnt, a) ordered so Pool's rounds (all ready the
            # moment params land) sit at the head of its queue; engine maps
            # balance ACT ~12.5us / Pool ~12us / DVE ~13.5us.
            UNITS = [(0, 1, 0), (0, 1, 1), (1, 1, 0), (1, 1, 1),
                     (2, 2, 0), (2, 2, 1), (4, 2, 0), (4, 2, 1),
                     (6, 1, 0), (6, 1, 1), (7, 1, 0), (7, 1, 1)]
            POOL_ROUND = {4, 5, 10, 11}    # indices into UNITS
            POOL_DEQ = set()
            ACT_DEQ = {4, 9, 11}
            for ui, (n0, cnt, a) in enumerate(UNITS):
                k = kp.tile([P, cnt, HW], I16, tag=f"k{cnt}",
                            bufs=(8 if cnt == 1 else 4))
                if ui in POOL_ROUND:
                    nc.gpsimd.tensor_scalar(k[:, :, :],
                                            out1[a][:, n0:n0 + cnt, :],
                                            A2[:, a:a + 1], B2[:, a:a + 1],
                                            OP.mult, OP.add)
                else:
                    nc.scalar.activation(k[:, :, :],
                                         out1[a][:, n0:n0 + cnt, :],
                                         ACTF, bias=B2[:, a:a + 1],
                                         scale=A2[:, a:a + 1])
                u = k.bitcast(F16)
                if ui in ACT_DEQ:
                    nc.scalar.activation(u[:, :, :], k[:, :, :], ACTF,
                                         bias=ymin2, scale=sy2)
                else:
                    ueng = nc.gpsimd if ui in POOL_DEQ else nc.vector
                    ueng.tensor_scalar(u[:, :, :], k[:, :, :],
                                       sy2, ymin2, OP.mult, OP.add)
                if voff + cnt * HW > 16384:
                    voff = 0
                v = xv[:, voff:voff + cnt * HW]
                voff += cnt * HW
                nc.vector.tensor_tensor(
                    v.rearrange("p (n y x) -> p n y x", y=32, x=32),
                    u.rearrange("p n (y x) -> p n y x", x=32),
                    qxpad[:, a * NLOC + n0:a * NLOC + n0 + cnt, 1:33, 1:33],
                    OP.add)
                nc.sync.dma_start(
                    out_d.ap()[n0:n0 + cnt, a * P:(a + 1) * P, :]
                    .rearrange("n c h -> c n h"),
                    v.rearrange("p (n h) -> p n h", n=cnt))

    nc.compile()
    return nc


def kernel(**inputs):
    global _cached_nc
    from concourse import bass_utils

    x = np.ascontiguousarray(
        np.asarray(inputs["x"], dtype=np.float32).reshape(64, C, HW)
        .astype(np.float16))
    w1 = np.asarray(inputs["w1"], dtype=np.float32).reshape(C, C, 9)
    w2 = np.asarray(inputs["w2"], dtype=np.float32).reshape(C, C, 9)
    # [co, ci, q] -> [ci, q, co], fp16
    w1t = np.ascontiguousarray(w1.transpose(1, 2, 0).astype(np.float16))
    w2t = np.ascontiguousarray(w2.transpose(1, 2, 0).astype(np.float16))
    g1 = np.ascontiguousarray(np.asarray(inputs["gamma1"], dtype=np.float32))
    b1 = np.ascontiguousarray(np.asarray(inputs["beta1"], dtype=np.float32))
    g2 = np.ascontiguousarray(np.asarray(inputs["gamma2"], dtype=np.float32))
    b2 = np.ascontiguousarray(np.asarray(inputs["beta2"], dtype=np.float32))

    if _cached_nc is None:
        _cached_nc = _build()
    nc = _cached_nc

    in_maps = []
    for cid in range(N_CORES):
        in_maps.append({
            "x": np.ascontiguousarray(x[cid * NLOC:(cid + 1) * NLOC]),
            "w1t": w1t, "w2t": w2t,
            "gamma1": g1, "beta1": b1, "gamma2": g2, "beta2": b2,
        })
    res = bass_utils.run_bass_kernel_spmd(
        nc, in_maps, core_ids=list(range(N_CORES)))
    out = np.concatenate(
        [res.results[cid]["out"].reshape(NLOC, C, 32, 32).astype(np.float32)
         for cid in range(N_CORES)],
        axis=0)
    kernel.last_results = res
    return out
